# revision 2
# baseline (speedup 1.0000x reference)
"""GCN (3x GCNConv + BN + final linear) on 8 TRN2 NeuronCores.

Strategy (see test.py for the harness):
- Pad N=50000 -> NP=50176 = 392 blocks of 128 nodes. Core c owns 49
  blocks (6272 nodes) and all edges whose destination (col) lies in them.
- The GCN norm dinv[row]*dinv[col] is factorized: dinv[row] is folded into
  edge_attr (host) and into the gather table rows (device); dinv[col] is
  applied to the aggregated block output (device).
- BatchNorm+bias are affine per-feature and are folded into the next
  layer's weights on device, so the edge phase is just
  msg = relu(table[row] + ea' @ We); out[col] += msg,
  computed as dense matmuls: per 128-edge chunk an indicator one-hot
  matrix S (built on DVE from uint8 compares) scatters messages into a
  PSUM accumulator per destination block; gathers use the hardware
  dma_gather (int16 indices, table split in two 25088-row halves).
- Per layer: node linear (feature-major) -> AllGather bf16 table ->
  edge phase -> BN stats AllReduce -> fold affine into next weights.
"""

import sys

sys.path.insert(0, "/opt/trn_rl_repo")

import numpy as np
import ml_dtypes

import concourse.bass as bass
import concourse.tile as tile
from concourse import bacc, mybir
from concourse.bass_utils import run_bass_kernel_spmd

# ---------------- constants ----------------
NCORES = 8
D = 128
DE = 16
EPS = 1e-5
P = 128


def configure(n):
    """Set the node count; everything else derives from it."""
    global N, BLOCKS, NP, BPC, NSH, VHALF
    N = n
    BLOCKS = ((N + P - 1) // P + NCORES - 1) // NCORES * NCORES
    NP = BLOCKS * P
    BPC = BLOCKS // NCORES
    NSH = BPC * P
    VHALF = NP // 2


configure(50000)
E = 1_600_000

dt = mybir.dt
AF = mybir.ActivationFunctionType
ALU = mybir.AluOpType

S_DTYPE = dt.bfloat16        # indicator matrix dtype


def _bf16(a):
    return np.asarray(a, dtype=np.float32).astype(ml_dtypes.bfloat16)


# ---------------- host-side edge preprocessing ----------------

def _preprocess(edge_index, edge_attr):
    """Sort/pad edges per (core, dest-block); build packed device arrays.

    Returns dict of per-core numpy arrays + the uniform chunk schedule.
    """
    row = np.asarray(edge_index[0], dtype=np.int64)
    col = np.asarray(edge_index[1], dtype=np.int64)
    deg = np.bincount(row, minlength=N).astype(np.float32) + 1.0
    dinv = deg ** -0.5                                  # [N]
    ea_s = np.asarray(edge_attr, np.float32) * dinv[row][:, None]   # [E,16]

    blk = col // P                                      # dest block of each edge
    order = np.argsort(blk, kind="stable")
    row_s, col_s, blk_s = row[order], col[order], blk[order]
    ea_sorted = ea_s[order]
    # boundaries per block
    starts = np.searchsorted(blk_s, np.arange(BLOCKS))
    ends = np.searchsorted(blk_s, np.arange(BLOCKS), side="right")

    # per (core, local block): split lo/hi rows, sizes
    lists = [[None] * BPC for _ in range(NCORES)]
    n_lo = np.zeros((NCORES, BPC), np.int64)
    n_hi = np.zeros((NCORES, BPC), np.int64)
    for g in range(BLOCKS):
        c, b = divmod(g, BPC)
        s, e = starts[g], ends[g]
        r = row_s[s:e]
        lo_mask = r < VHALF
        lo_idx = np.nonzero(lo_mask)[0]
        hi_idx = np.nonzero(~lo_mask)[0]
        lists[c][b] = (s, lo_idx, hi_idx)
        n_lo[c, b] = len(lo_idx)
        n_hi[c, b] = len(hi_idx)

    # uniform chunk counts across cores
    m_lo = np.maximum(1, (n_lo.max(axis=0) + P - 1) // P).astype(int)   # [BPC]
    m_hi = np.maximum(1, (n_hi.max(axis=0) + P - 1) // P).astype(int)
    m_tot = m_lo + m_hi
    tot_chunks = int(m_tot.sum())
    chunk_off = np.zeros(BPC, int)
    chunk_off[1:] = np.cumsum(m_tot)[:-1]
    # idx columns (16-wrapped) offsets, in units of int16 columns
    s_lo = m_lo * 8
    s_hi = m_hi * 8
    s_tot = s_lo + s_hi
    tot_s = int(s_tot.sum())
    s_off = np.zeros(BPC, int)
    s_off[1:] = np.cumsum(s_tot)[:-1]

    per_core = []
    for c in range(NCORES):
        eaT = np.zeros((DE, tot_chunks, P), np.float32)
        colrel = np.full((P, tot_chunks), 255, np.uint8)
        idx16 = np.zeros((16, tot_s), np.int16)
        for b in range(BPC):
            s, lo_idx, hi_idx = lists[c][b]
            co = chunk_off[b]
            for half, sub, m_half, half_chunk_base, base in (
                (0, lo_idx, m_lo[b], 0, 0),
                (1, hi_idx, m_hi[b], m_lo[b], VHALF),
            ):
                g_sz = int(m_half) * P
                rows_h = np.zeros(g_sz, np.int64)        # pad idx -> 0
                rows_h[: len(sub)] = row_s[s + sub] - base
                cols_h = np.full(g_sz, 255, np.int64)    # pad col -> 255
                cols_h[: len(sub)] = col_s[s + sub] - (c * BPC + b) * P
                ea_h = np.zeros((g_sz, DE), np.float32)
                ea_h[: len(sub)] = ea_sorted[s + sub]
                ii = np.arange(g_sz)
                pp, jj = ii % P, ii // P
                eaT[:, co + half_chunk_base + jj, pp] = ea_h.T
                colrel[pp, co + half_chunk_base + jj] = cols_h
                # 16-wrapped idx at column offset
                so = s_off[b] + (0 if half == 0 else s_lo[b])
                idx16[ii % 16, so + ii // 16] = rows_h
        per_core.append(
            dict(
                eaT=_bf16(eaT),
                colrel=colrel,
                idx16=np.tile(idx16, (8, 1)),            # replicate to 128 partitions
            )
        )

    sched = dict(
        m_lo=[int(v) for v in m_lo], m_hi=[int(v) for v in m_hi],
        chunk_off=[int(v) for v in chunk_off], s_off=[int(v) for v in s_off],
        s_lo=[int(v) for v in s_lo],
        tot_chunks=tot_chunks, tot_s=tot_s,
    )
    return per_core, sched, dinv


# ---------------- device program ----------------

def _build(sched):
    nc = bacc.Bacc(None, target_bir_lowering=False, debug=False)
    TC, TS = sched["tot_chunks"], sched["tot_s"]

    # ---- external inputs (per-core shapes) ----
    decl = nc.declare_dram_parameter
    xT = decl("xT", [P, NSH], dt.bfloat16, isOutput=False)
    eaT_d = decl("eaT", [DE, TC, P], dt.bfloat16, isOutput=False)
    colrel_d = decl("colrel", [P, TC], dt.uint8, isOutput=False)
    idx_d = decl("idx16", [P, TS], dt.int16, isOutput=False)
    dinv_d = decl("dinvt", [P, NSH], dt.bfloat16, isOutput=False)
    iota_d = decl("iota_u8", [P, P], dt.uint8, isOutput=False)
    ident_d = decl("ident", [P, P], dt.bfloat16, isOutput=False)
    W_d = [decl(f"W{k}", [D, D], dt.bfloat16, isOutput=False) for k in (1, 2, 3)]
    Wf_d = [decl(f"Wf{k}", [D, D], dt.float32, isOutput=False) for k in (2, 3)]
    Wl_d = decl("Wlin", [D, D], dt.bfloat16, isOutput=False)
    Wlf_d = decl("Wlinf", [D, D], dt.float32, isOutput=False)
    We_d = [decl(f"We{k}", [DE, D], dt.bfloat16, isOutput=False) for k in (1, 2, 3)]
    # b_tot[k] = b_k + be_k as column [128,1]; rows [1,128] for fold matmuls
    bcol1_d = decl("bcol1", [D, 1], dt.float32, isOutput=False)
    brow_d = [decl(f"brow{k}", [1, D], dt.float32, isOutput=False) for k in (2, 3)]
    blrow_d = decl("blrow", [1, D], dt.float32, isOutput=False)
    g_d = [decl(f"g{k}", [D, 1], dt.float32, isOutput=False) for k in (1, 2, 3)]
    bt_d = [decl(f"bt{k}", [D, 1], dt.float32, isOutput=False) for k in (1, 2, 3)]
    outT = decl("outT", [P, NSH], dt.float32, isOutput=True)
    import os
    DBG = os.environ.get("KDBG") == "1"
    SKIP_GATHER = os.environ.get("KSKIP_GATHER") == "1"
    SKIP_CC = os.environ.get("KSKIP_CC") == "1"
    SKIP_EDGE = os.environ.get("KSKIP_EDGE") == "1"
    SKIP_EA = os.environ.get("KSKIP_EA") == "1"
    if DBG:
        dbg_table = decl("dbg_table", [NP, D], dt.bfloat16, isOutput=True)
        dbg_hr = decl("dbg_hr", [P, 4 * D], dt.bfloat16, isOutput=True)
        dbg_msg = decl("dbg_msg", [P, 4 * D], dt.bfloat16, isOutput=True)
        dbg_t1 = decl("dbg_t1", [P, 6 * P], dt.bfloat16, isOutput=True)
        dbg_stats = decl("dbg_stats", [P, 6], dt.float32, isOutput=True)
        dbg_table2 = decl("dbg_table2", [NP, D], dt.bfloat16, isOutput=True)

    rg = [list(range(NCORES))]

    with tile.TileContext(nc) as tc:
        import contextlib
        with contextlib.ExitStack() as ctx:
            ek = ctx.enter_context
            const = ek(tc.tile_pool(name="const", bufs=1))
            nodeb = ek(tc.tile_pool(name="nodeb", bufs=2))
            edge_ea = ek(tc.tile_pool(name="edge_ea", bufs=2))
            edge_idx = ek(tc.tile_pool(name="edge_idx", bufs=3))
            edge_hr = ek(tc.tile_pool(name="edge_hr", bufs=2))
            edge_msg = ek(tc.tile_pool(name="edge_msg", bufs=2))
            edge_S = ek(tc.tile_pool(name="edge_S", bufs=2))
            small = ek(tc.tile_pool(name="small", bufs=4))
            trp = ek(tc.tile_pool(name="trp", bufs=3))
            ps_mp = ek(tc.tile_pool(name="ps_mp", bufs=3, space="PSUM"))
            ps_conv = ek(tc.tile_pool(name="ps_conv", bufs=2, space="PSUM"))
            ps_node = ek(tc.tile_pool(name="ps_node", bufs=1, space="PSUM"))
            ps_tr = ek(tc.tile_pool(name="ps_tr", bufs=1, space="PSUM"))
            dram = ek(tc.tile_pool(name="dram", bufs=2, space="DRAM"))

            # ---- load constants ----
            def ld(pool, shape, dty, src, name):
                t = pool.tile(shape, dty, name=name)
                nc.sync.dma_start(out=t[:], in_=src[...])
                return t

            xT_t = ld(const, [P, NSH], dt.bfloat16, xT, 'xT_t')
            dinv_t = ld(const, [P, NSH], dt.bfloat16, dinv_d, 'dinv_d_t')
            iota_t = ld(const, [P, P], dt.uint8, iota_d, 'iota_d_t')
            ident_t = ld(const, [P, P], dt.bfloat16, ident_d, 'ident_d_t')
            colrel_t = ld(const, [P, TC], dt.uint8, colrel_d, 'colrel_d_t')
            W_t = [ld(const, [D, D], dt.bfloat16, W_d[i], f'W_t{i}') for i in range(3)]
            Wf_t = [ld(const, [D, D], dt.float32, Wf_d[i], f'Wf_t{i}') for i in range(2)]
            Wl_t = ld(const, [D, D], dt.bfloat16, Wl_d, 'Wl_d_t')
            Wlf_t = ld(const, [D, D], dt.float32, Wlf_d, 'Wlf_d_t')
            We_t = [ld(const, [DE, D], dt.bfloat16, We_d[i], f'We_t{i}') for i in range(3)]
            bcol1_t = ld(const, [D, 1], dt.float32, bcol1_d, 'bcol1_d_t')
            brow_t = [ld(const, [1, D], dt.float32, brow_d[i], f'brow_t{i}') for i in range(2)]
            blrow_t = ld(const, [1, D], dt.float32, blrow_d, 'blrow_d_t')
            g_t = [ld(const, [D, 1], dt.float32, g_d[i], f'g_t{i}') for i in range(3)]
            bt_t = [ld(const, [D, 1], dt.float32, bt_d[i], f'bt_t{i}') for i in range(3)]

            t_T = [const.tile([P, NSH], dt.bfloat16, name=f't_T{i}') for i in range(2)]
            eps_t = const.tile([P, 1], dt.float32, name='eps_t')
            nc.vector.memset(eps_t[:], EPS)

            m_lo, m_hi = sched["m_lo"], sched["m_hi"]
            chunk_off, s_off, s_lo = sched["chunk_off"], sched["s_off"], sched["s_lo"]

            col_chunks = [(o, min(512, NSH - o)) for o in range(0, NSH, 512)]

            def node_linear(rhs_t, Wp_t, bp_t, hout_t):
                """hout = dinv * (W'.T @ rhs + b')  (feature-major)."""
                for (o, w) in col_chunks:
                    pp = ps_node.tile([P, w], dt.float32, space="PSUM",
                                         padded_shape=[P, 512])
                    nc.tensor.matmul(out=pp[:], lhsT=Wp_t[:], rhs=rhs_t[:, o:o + w],
                                     start=True, stop=True)
                    tmp = trp.tile([P, w], dt.bfloat16)
                    nc.scalar.activation(out=tmp[:], in_=pp[:], func=AF.Identity,
                                         bias=bp_t[:])
                    nc.vector.tensor_tensor(out=hout_t[:, o:o + w], in0=tmp[:],
                                            in1=dinv_t[:, o:o + w], op=ALU.mult)

            for k in range(3):  # layers 1..3
                # ---- fold previous BN (k>=1) into this layer's weights ----
                if k == 0:
                    Wp_t, bp_t = W_t[0], bcol1_t
                    rhs_t = xT_t
                else:
                    a_t, c_t = bn_fold  # from previous layer epilogue
                    Wp_t = small.tile([D, D], dt.bfloat16)
                    nc.scalar.activation(out=Wp_t[:], in_=W_t[k][:], func=AF.Identity,
                                         scale=a_t[:])
                    pb = ps_tr.tile([1, D], dt.float32, space="PSUM",
                                    padded_shape=[1, 512])
                    nc.tensor.matmul(out=pb[:], lhsT=c_t[:], rhs=Wf_t[k - 1][:],
                                     start=True, stop=True)
                    bprow = small.tile([1, D], dt.float32)
                    nc.vector.tensor_tensor(out=bprow[:], in0=pb[:], in1=brow_t[k - 1][:],
                                            op=ALU.add)
                    bp_bounce = dram.tile([1, D], dt.float32, name='bp_bounce')
                    nc.sync.dma_start(out=bp_bounce[:], in_=bprow[:])
                    bp_t = small.tile([D, 1], dt.float32)
                    nc.sync.dma_start(out=bp_t[:], in_=bp_bounce[0, :, None])
                    rhs_t = t_T[(k - 1) % 2]

                # ---- node linear + dinv scale ----
                hlin_t = nodeb.tile([P, NSH], dt.bfloat16)
                node_linear(rhs_t, Wp_t, bp_t, hlin_t)

                # ---- transpose tiles to node-major, write shard, AllGather ----
                shard = dram.tile([NSH, D], dt.bfloat16)
                table = dram.tile([NP, D], dt.bfloat16)
                for t in range(BPC):
                    ptr = ps_tr.tile([P, P], dt.bfloat16, space="PSUM",
                                     padded_shape=[P, 1024])
                    nc.tensor.transpose(out=ptr[:], in_=hlin_t[:, t * P:(t + 1) * P],
                                        identity=ident_t[:])
                    sb = trp.tile([P, P], dt.bfloat16)
                    nc.scalar.activation(out=sb[:], in_=ptr[:], func=AF.Copy)
                    nc.sync.dma_start(out=shard[t * P:(t + 1) * P, :], in_=sb[:])
                if not SKIP_CC:
                    nc.gpsimd.collective_compute(
                        "AllGather", ALU.bypass, replica_groups=rg,
                        ins=[shard[:].opt()], outs=[table[:].opt()],
                    )
                else:
                    nc.sync.dma_start(out=table[:NSH, :], in_=shard[:, :])
                if DBG and k == 0:
                    nc.sync.dma_start(out=dbg_table[:, :], in_=table[:, :])
                if DBG and k == 1:
                    nc.sync.dma_start(out=dbg_table2[:, :], in_=table[:, :])

                # ---- edge phase over 49 dest blocks ----
                sums_t = small.tile([P, BPC], dt.float32)
                sqs_t = small.tile([P, BPC], dt.float32)
                tnew = t_T[k % 2]
                for b in range(BPC):
                    m = m_lo[b] + m_hi[b]
                    co = chunk_off[b]
                    ea_t = edge_ea.tile([DE, m, P], dt.bfloat16)
                    if not SKIP_EA:
                        nc.sync.dma_start(out=ea_t[:], in_=eaT_d[:, co:co + m, :])
                    else:
                        nc.vector.memset(ea_t[:, :1, :], 0.0)
                    stot_b = s_lo[b] + m_hi[b] * 8
                    idx_t = edge_idx.tile([P, stot_b], dt.int16, name=f'idxb')
                    nc.sync.dma_start(out=idx_t[:], in_=idx_d[:, s_off[b]:s_off[b] + stot_b])
                    hr_t = edge_hr.tile([P, m, D], dt.bfloat16)
                    # dma_gather's Q7 ucode handles at most 1024 indices
                    # per call -> split into <=8-chunk (1024-edge) pieces.
                    if not SKIP_GATHER:
                        for mh, tab, mbase, sbase in (
                            (m_lo[b], table[:VHALF, :], 0, 0),
                            (m_hi[b], table[VHALF:, :], m_lo[b], s_lo[b]),
                        ):
                            for pc in range(0, mh, 8):
                                pw = min(8, mh - pc)
                                nc.gpsimd.dma_gather(
                                    out_ap=hr_t[:, mbase + pc:mbase + pc + pw, :],
                                    in_ap=tab,
                                    idxs_ap=idx_t[:, sbase + pc * 8:sbase + (pc + pw) * 8],
                                    num_idxs=pw * P, num_idxs_reg=pw * P, elem_size=D,
                                )
                    else:
                        nc.vector.memset(hr_t[:, :1, :], 0.25)
                    if DBG and k == 0 and b == 0:
                        dbg_hr_sb = trp.tile([P, 4 * D], dt.bfloat16, name='dbg_hr_sb')
                        nc.vector.tensor_copy(out=dbg_hr_sb[:], in_=hr_t[:, :4, :].rearrange("p m d -> p (m d)"))
                        nc.sync.dma_start(out=dbg_hr[:, :], in_=dbg_hr_sb[:])
                    # S indicator [P, m, P]
                    S_t = edge_S.tile([P, m, P], S_DTYPE)
                    iota_b = bass.AP(tensor=iota_t.tensor, offset=iota_t[:].offset,
                                     ap=[iota_t[:].ap[0], [0, m], iota_t[:].ap[1]])
                    cr = colrel_t[:, co:co + m]
                    cr_b = bass.AP(tensor=colrel_t.tensor, offset=cr.offset,
                                   ap=[cr.ap[0], cr.ap[1], [0, P]])
                    nc.vector.tensor_tensor(out=S_t[:], in0=iota_b, in1=cr_b,
                                            op=ALU.is_equal)
                    # messages
                    msg_t = edge_msg.tile([P, m, D], dt.bfloat16)
                    if SKIP_EDGE:
                        nc.vector.memset(msg_t[:, :1, :], 0.1)
                    j = 0 if not SKIP_EDGE else m
                    while j < m:
                        jw = min(4, m - j)
                        mp = ps_mp.tile([P, 4, D], dt.float32, space="PSUM")
                        # start=True zeroes the whole 2KB bank, so the
                        # full-tile identity-add must come first.
                        nc.tensor.matmul(
                            out=mp[:, :jw, :].rearrange("p j d -> p (j d)"),
                            lhsT=ident_t[:],
                            rhs=hr_t[:, j:j + jw, :].rearrange("p j d -> p (j d)"),
                            start=True, stop=False, skip_group_check=True)
                        for jj in range(jw):
                            nc.tensor.matmul(out=mp[:, jj, :],
                                             lhsT=ea_t[:, j + jj, :], rhs=We_t[k][:],
                                             start=False, stop=(jj == jw - 1),
                                             skip_group_check=True)
                        nc.scalar.activation(
                            out=msg_t[:, j:j + jw, :].rearrange("p j d -> p (j d)"),
                            in_=mp[:, :jw, :].rearrange("p j d -> p (j d)"), func=AF.Relu)
                        j += jw
                    if DBG and k == 0 and b == 0:
                        dbg_msg_sb = trp.tile([P, 4 * D], dt.bfloat16, name='dbg_msg_sb')
                        nc.vector.tensor_copy(out=dbg_msg_sb[:], in_=msg_t[:, :4, :].rearrange("p m d -> p (m d)"))
                        nc.sync.dma_start(out=dbg_msg[:, :], in_=dbg_msg_sb[:])
                    # scatter into conv accumulator (feature-major out)
                    cp = ps_conv.tile([P, P], dt.float32, space="PSUM",
                                      padded_shape=[P, 512])
                    for j in range(m if not SKIP_EDGE else 1):
                        nc.tensor.matmul(out=cp[:], lhsT=msg_t[:, j, :],
                                         rhs=S_t[:, j, :],
                                         start=(j == 0), stop=(j == m - 1))
                    # epilogue: dinv scale, relu -> t, stats
                    sl = slice(b * P, (b + 1) * P)
                    pre = trp.tile([P, P], dt.float32)
                    nc.vector.tensor_tensor(out=pre[:], in0=cp[:],
                                            in1=dinv_t[:, sl], op=ALU.mult)
                    nc.scalar.activation(out=tnew[:, sl], in_=pre[:], func=AF.Relu,
                                         accum_out=sums_t[:, b:b + 1])
                    sq_scr = trp.tile([P, P], dt.bfloat16)
                    nc.scalar.activation(out=sq_scr[:], in_=tnew[:, sl], func=AF.Square,
                                         accum_out=sqs_t[:, b:b + 1])

                # ---- BN stats + fold coefficients ----
                st = small.tile([P, 2], dt.float32)
                nc.vector.tensor_reduce(out=st[:, 0:1], in_=sums_t[:],
                                        axis=mybir.AxisListType.X, op=ALU.add)
                nc.vector.tensor_reduce(out=st[:, 1:2], in_=sqs_t[:],
                                        axis=mybir.AxisListType.X, op=ALU.add)
                st_in = dram.tile([P, 2], dt.float32)
                st_out = dram.tile([P, 2], dt.float32)
                nc.sync.dma_start(out=st_in[:], in_=st[:])
                if not SKIP_CC:
                    nc.gpsimd.collective_compute(
                        "AllReduce", ALU.add, replica_groups=rg,
                        ins=[st_in[:].opt()], outs=[st_out[:].opt()],
                    )
                else:
                    nc.sync.dma_start(out=st_out[:, :], in_=st_in[:, :])
                stg = small.tile([P, 2], dt.float32)
                nc.sync.dma_start(out=stg[:], in_=st_out[:])
                mu = small.tile([P, 1], dt.float32)
                nc.vector.tensor_scalar(out=mu[:], in0=stg[:, 0:1], scalar1=1.0 / N,
                                        scalar2=None, op0=ALU.mult)
                ex2 = small.tile([P, 1], dt.float32)
                nc.vector.tensor_scalar(out=ex2[:], in0=stg[:, 1:2], scalar1=1.0 / N,
                                        scalar2=None, op0=ALU.mult)
                var = small.tile([P, 1], dt.float32)
                nc.vector.tensor_tensor(out=var[:], in0=mu[:], in1=mu[:], op=ALU.mult)
                nc.vector.tensor_tensor(out=var[:], in0=ex2[:], in1=var[:],
                                        op=ALU.subtract)
                sd = small.tile([P, 1], dt.float32)
                nc.scalar.activation(out=sd[:], in_=var[:], func=AF.Sqrt, bias=eps_t[:])
                rs = small.tile([P, 1], dt.float32)
                nc.vector.reciprocal(out=rs[:], in_=sd[:])
                a_t = small.tile([P, 1], dt.float32)
                nc.vector.tensor_tensor(out=a_t[:], in0=rs[:], in1=g_t[k][:],
                                        op=ALU.mult)
                c_t = small.tile([P, 1], dt.float32)
                nc.vector.tensor_tensor(out=c_t[:], in0=mu[:], in1=a_t[:], op=ALU.mult)
                nc.vector.tensor_tensor(out=c_t[:], in0=bt_t[k][:], in1=c_t[:],
                                        op=ALU.subtract)
                bn_fold = (a_t, c_t)
                if DBG and k == 0:
                    nt = min(6 * P, NSH)
                    dbg_t1_sb = trp.tile([P, 6 * P], dt.bfloat16, name='dbg_t1_sb')
                    nc.vector.tensor_copy(out=dbg_t1_sb[:, :nt], in_=tnew[:, :nt])
                    nc.sync.dma_start(out=dbg_t1[:, :], in_=dbg_t1_sb[:])
                    dbg_st_sb = trp.tile([P, 6], dt.float32, name='dbg_st_sb')
                    for ii, tt in enumerate((mu, ex2, var, sd, a_t, c_t)):
                        nc.vector.tensor_copy(out=dbg_st_sb[:, ii:ii+1], in_=tt[:])
                    nc.sync.dma_start(out=dbg_stats[:, :], in_=dbg_st_sb[:])

            # ---- final linear: out^T = Wl'.T @ t3 + bl' ----
            a_t, c_t = bn_fold
            Wlp = small.tile([D, D], dt.bfloat16)
            nc.scalar.activation(out=Wlp[:], in_=Wl_t[:], func=AF.Identity, scale=a_t[:])
            pb = ps_tr.tile([1, D], dt.float32, space="PSUM",
                            padded_shape=[1, 512])
            nc.tensor.matmul(out=pb[:], lhsT=c_t[:], rhs=Wlf_t[:], start=True, stop=True)
            blp_row = small.tile([1, D], dt.float32)
            nc.vector.tensor_tensor(out=blp_row[:], in0=pb[:], in1=blrow_t[:], op=ALU.add)
            blp_bounce = dram.tile([1, D], dt.float32, name='blp_bounce')
            nc.sync.dma_start(out=blp_bounce[:], in_=blp_row[:])
            blp = small.tile([D, 1], dt.float32)
            nc.sync.dma_start(out=blp[:], in_=blp_bounce[0, :, None])
            for (o, w) in col_chunks:
                pp = ps_node.tile([P, w], dt.float32, space="PSUM",
                                     padded_shape=[P, 512])
                nc.tensor.matmul(out=pp[:], lhsT=Wlp[:], rhs=t_T[0][:, o:o + w],
                                 start=True, stop=True)
                ot = trp.tile([P, w], dt.float32)
                nc.scalar.activation(out=ot[:], in_=pp[:], func=AF.Identity, bias=blp[:])
                nc.sync.dma_start(out=outT[:, o:o + w], in_=ot[:])

    nc.finalize()
    return nc


# ---------------- public entry point ----------------

_CACHE = {}
LAST_EXEC_NS = None


def _make_in_maps(inputs, per_core, dinv):
    x = np.asarray(inputs["x"], np.float32)

    dinv_pad = np.zeros(NP, np.float32)
    dinv_pad[:N] = dinv
    xT_full = np.zeros((P, NP), np.float32)
    xT_full[:, :N] = x.T

    Ws = {k: np.asarray(inputs[k], np.float32) for k in
          ("W1", "W2", "W3", "Wl", "We1", "We2", "We3")}
    bt_tot = {k: np.asarray(inputs[f"b{k}"], np.float32) +
                 np.asarray(inputs[f"be{k}"], np.float32) for k in (1, 2, 3)}

    in_maps = []
    for c in range(NCORES):
        sl = slice(c * NSH, (c + 1) * NSH)
        im = dict(per_core[c])
        im["xT"] = _bf16(xT_full[:, sl])
        im["dinvt"] = _bf16(np.tile(dinv_pad[sl][None, :], (P, 1)))
        im["iota_u8"] = np.tile(np.arange(P, dtype=np.uint8)[None, :], (P, 1))
        im["ident"] = _bf16(np.eye(P))
        for i, k in enumerate((1, 2, 3)):
            im[f"W{k}"] = _bf16(Ws[f"W{k}"])
            im[f"We{k}"] = _bf16(Ws[f"We{k}"])
            im[f"g{k}"] = np.asarray(inputs[f"g{k}"], np.float32).reshape(D, 1)
            im[f"bt{k}"] = np.asarray(inputs[f"bt{k}"], np.float32).reshape(D, 1)
        im["Wf2"] = Ws["W2"]
        im["Wf3"] = Ws["W3"]
        im["Wlin"] = _bf16(Ws["Wl"])
        im["Wlinf"] = Ws["Wl"]
        im["bcol1"] = bt_tot[1].reshape(D, 1)
        im["brow2"] = bt_tot[2].reshape(1, D)
        im["brow3"] = bt_tot[3].reshape(1, D)
        im["blrow"] = np.asarray(inputs["bl"], np.float32).reshape(1, D)
        in_maps.append(im)
    return in_maps


def kernel(**inputs):
    edge_attr = np.asarray(inputs["edge_attr"], np.float32)
    edge_index = np.asarray(inputs["edge_index"])

    per_core, sched, dinv = _preprocess(edge_index, edge_attr)
    in_maps = _make_in_maps(inputs, per_core, dinv)

    key = ("k", sched["tot_chunks"], sched["tot_s"],
           tuple(sched["m_lo"]), tuple(sched["m_hi"]))
    if key not in _CACHE:
        _CACHE[key] = _build(sched)
    nc = _CACHE[key]

    import os
    trace = os.environ.get("KPROF") == "1"
    r = run_bass_kernel_spmd(nc, in_maps, core_ids=list(range(NCORES)), trace=trace)
    if trace:
        print(f"HW exec time: {r.exec_time_ns} ns", flush=True)
        global LAST_EXEC_NS
        LAST_EXEC_NS = r.exec_time_ns
        try:
            insts = r.instructions_and_trace[0] if r.instructions_and_trace else []
            import pickle
            rows = [
                dict(ts=i.timestamp, dur=i.duration, engine=str(i.engine),
                     name=i.name, label=i.label, wait=i.evt_wait_time,
                     bir=str(i.bir_str)[:200], src=f"{i.source_file}:{i.source_line}")
                for i in insts
            ]
            with open("/tmp/kprof_insts.pkl", "wb") as f:
                pickle.dump(rows, f)
            print(f"KPROF: dumped {len(rows)} insts; trace="
                  f"{r.instructions_and_trace[1] if r.instructions_and_trace else None}",
                  flush=True)
        except Exception as e:
            print(f"KPROF dump failed: {e}", flush=True)
    res = r.results
    outT = np.concatenate([res[c]["outT"] for c in range(NCORES)], axis=1)  # [128, NP]
    return np.ascontiguousarray(outT.T[:N]).astype(np.float32)



# revision 5
# speedup vs baseline: 2.3020x; 2.3020x over previous
"""GCN (3x GCNConv + BN + final linear) on 8 TRN2 NeuronCores.

Strategy (see test.py for the harness):
- Pad N=50000 -> NP=50176 = 392 blocks of 128 nodes. Core c owns 49
  blocks (6272 nodes) and all edges whose destination (col) lies in them.
- The GCN norm dinv[row]*dinv[col] is factorized: dinv[row] is folded into
  edge_attr (host) and into the gather table rows (device); dinv[col] is
  applied to the aggregated block output (device).
- BatchNorm+bias are affine per-feature and are folded into the next
  layer's weights on device, so the edge phase is just
  msg = relu(table[row] + ea' @ We); out[col] += msg,
  computed as dense matmuls: per 128-edge chunk an indicator one-hot
  matrix S (built on DVE from uint8 compares) scatters messages into a
  PSUM accumulator per destination block; gathers use the hardware
  dma_gather (int16 indices, table split in two 25088-row halves).
- Per layer: node linear (feature-major) -> AllGather bf16 table ->
  edge phase -> BN stats AllReduce -> fold affine into next weights.
"""

import sys

sys.path.insert(0, "/opt/trn_rl_repo")

import numpy as np
import ml_dtypes

import concourse.bass as bass
import concourse.tile as tile
from concourse import bacc, mybir
from concourse.bass_utils import run_bass_kernel_spmd

# ---------------- constants ----------------
NCORES = 8
D = 128
DE = 16
EPS = 1e-5
P = 128


def configure(n):
    """Set the node count; everything else derives from it."""
    global N, BLOCKS, NP, BPC, NSH, VHALF
    N = n
    BLOCKS = ((N + P - 1) // P + NCORES - 1) // NCORES * NCORES
    NP = BLOCKS * P
    BPC = BLOCKS // NCORES
    NSH = BPC * P
    VHALF = NP // 2


configure(50000)
E = 1_600_000

dt = mybir.dt
AF = mybir.ActivationFunctionType
ALU = mybir.AluOpType

S_DTYPE = dt.bfloat16        # indicator matrix dtype


def _bf16(a):
    return np.asarray(a, dtype=np.float32).astype(ml_dtypes.bfloat16)


# ---------------- host-side edge preprocessing ----------------

def _preprocess(edge_index, edge_attr):
    """Sort/pad edges per (core, dest-block); build packed device arrays.

    Returns dict of per-core numpy arrays + the uniform chunk schedule.
    """
    row = np.asarray(edge_index[0], dtype=np.int64)
    col = np.asarray(edge_index[1], dtype=np.int64)
    deg = np.bincount(row, minlength=N).astype(np.float32) + 1.0
    dinv = deg ** -0.5                                  # [N]
    ea_s = np.asarray(edge_attr, np.float32) * dinv[row][:, None]   # [E,16]

    blk = col // P                                      # dest block of each edge
    order = np.argsort(blk, kind="stable")
    row_s, col_s, blk_s = row[order], col[order], blk[order]
    ea_sorted = ea_s[order]
    # boundaries per block
    starts = np.searchsorted(blk_s, np.arange(BLOCKS))
    ends = np.searchsorted(blk_s, np.arange(BLOCKS), side="right")

    # per (core, local block): split lo/hi rows, sizes
    lists = [[None] * BPC for _ in range(NCORES)]
    n_lo = np.zeros((NCORES, BPC), np.int64)
    n_hi = np.zeros((NCORES, BPC), np.int64)
    for g in range(BLOCKS):
        c, b = divmod(g, BPC)
        s, e = starts[g], ends[g]
        r = row_s[s:e]
        lo_mask = r < VHALF
        lo_idx = np.nonzero(lo_mask)[0]
        hi_idx = np.nonzero(~lo_mask)[0]
        lists[c][b] = (s, lo_idx, hi_idx)
        n_lo[c, b] = len(lo_idx)
        n_hi[c, b] = len(hi_idx)

    # uniform chunk counts across cores
    m_lo = np.maximum(1, (n_lo.max(axis=0) + P - 1) // P).astype(int)   # [BPC]
    m_hi = np.maximum(1, (n_hi.max(axis=0) + P - 1) // P).astype(int)
    m_tot = m_lo + m_hi
    tot_chunks = int(m_tot.sum())
    chunk_off = np.zeros(BPC, int)
    chunk_off[1:] = np.cumsum(m_tot)[:-1]
    # idx columns (16-wrapped) offsets, in units of int16 columns
    s_lo = m_lo * 8
    s_hi = m_hi * 8
    s_tot = s_lo + s_hi
    tot_s = int(s_tot.sum())
    s_off = np.zeros(BPC, int)
    s_off[1:] = np.cumsum(s_tot)[:-1]

    per_core = []
    for c in range(NCORES):
        eaT = np.zeros((DE, tot_chunks, P), np.float32)
        colrel = np.full((P, tot_chunks), 255, np.uint8)
        idx16 = np.zeros((16, tot_s), np.int16)
        for b in range(BPC):
            s, lo_idx, hi_idx = lists[c][b]
            co = chunk_off[b]
            for half, sub, m_half, half_chunk_base, base in (
                (0, lo_idx, m_lo[b], 0, 0),
                (1, hi_idx, m_hi[b], m_lo[b], VHALF),
            ):
                g_sz = int(m_half) * P
                rows_h = np.zeros(g_sz, np.int64)        # pad idx -> 0
                rows_h[: len(sub)] = row_s[s + sub] - base
                cols_h = np.full(g_sz, 255, np.int64)    # pad col -> 255
                cols_h[: len(sub)] = col_s[s + sub] - (c * BPC + b) * P
                ea_h = np.zeros((g_sz, DE), np.float32)
                ea_h[: len(sub)] = ea_sorted[s + sub]
                ii = np.arange(g_sz)
                pp, jj = ii % P, ii // P
                eaT[:, co + half_chunk_base + jj, pp] = ea_h.T
                colrel[pp, co + half_chunk_base + jj] = cols_h
                # 16-wrapped idx at column offset
                so = s_off[b] + (0 if half == 0 else s_lo[b])
                idx16[ii % 16, so + ii // 16] = rows_h
        per_core.append(
            dict(
                eaT=_bf16(eaT),
                colrel=colrel,
                idx16=np.tile(idx16, (8, 1)),            # replicate to 128 partitions
            )
        )

    sched = dict(
        m_lo=[int(v) for v in m_lo], m_hi=[int(v) for v in m_hi],
        chunk_off=[int(v) for v in chunk_off], s_off=[int(v) for v in s_off],
        s_lo=[int(v) for v in s_lo],
        tot_chunks=tot_chunks, tot_s=tot_s,
    )
    return per_core, sched, dinv


# ---------------- device program ----------------

NQ = 4  # SWDGE queues: dma_gather queue q runs on Q7 core pair (2q, 2q+1),
        # so round-robin over 4 queues runs 4 gathers concurrently.


def _build(sched):
    nc = bacc.Bacc(None, target_bir_lowering=False, debug=False,
                   num_swdge_queues=NQ)
    TC, TS = sched["tot_chunks"], sched["tot_s"]

    # ---- external inputs (per-core shapes) ----
    decl = nc.declare_dram_parameter
    xT = decl("xT", [P, NSH], dt.bfloat16, isOutput=False)
    eaT_d = decl("eaT", [DE, TC, P], dt.bfloat16, isOutput=False)
    colrel_d = decl("colrel", [P, TC], dt.uint8, isOutput=False)
    idx_d = decl("idx16", [P, TS], dt.int16, isOutput=False)
    dinv_d = decl("dinvt", [P, NSH], dt.bfloat16, isOutput=False)
    iota_d = decl("iota_u8", [P, P], dt.uint8, isOutput=False)
    ident_d = decl("ident", [P, P], dt.bfloat16, isOutput=False)
    W_d = [decl(f"W{k}", [D, D], dt.bfloat16, isOutput=False) for k in (1, 2, 3)]
    Wf_d = [decl(f"Wf{k}", [D, D], dt.float32, isOutput=False) for k in (2, 3)]
    Wl_d = decl("Wlin", [D, D], dt.bfloat16, isOutput=False)
    Wlf_d = decl("Wlinf", [D, D], dt.float32, isOutput=False)
    We_d = [decl(f"We{k}", [DE, D], dt.bfloat16, isOutput=False) for k in (1, 2, 3)]
    # b_tot[k] = b_k + be_k as column [128,1]; rows [1,128] for fold matmuls
    bcol1_d = decl("bcol1", [D, 1], dt.float32, isOutput=False)
    brow_d = [decl(f"brow{k}", [1, D], dt.float32, isOutput=False) for k in (2, 3)]
    blrow_d = decl("blrow", [1, D], dt.float32, isOutput=False)
    g_d = [decl(f"g{k}", [D, 1], dt.float32, isOutput=False) for k in (1, 2, 3)]
    bt_d = [decl(f"bt{k}", [D, 1], dt.float32, isOutput=False) for k in (1, 2, 3)]
    outT = decl("outT", [P, NSH], dt.float32, isOutput=True)
    import os
    DBG = os.environ.get("KDBG") == "1"
    SKIP_GATHER = os.environ.get("KSKIP_GATHER") == "1"
    SKIP_CC = os.environ.get("KSKIP_CC") == "1"
    SKIP_EDGE = os.environ.get("KSKIP_EDGE") == "1"
    SKIP_EA = os.environ.get("KSKIP_EA") == "1"
    if DBG:
        dbg_table = decl("dbg_table", [NP, D], dt.bfloat16, isOutput=True)
        dbg_hr = decl("dbg_hr", [P, 4 * D], dt.bfloat16, isOutput=True)
        dbg_msg = decl("dbg_msg", [P, 4 * D], dt.bfloat16, isOutput=True)
        dbg_t1 = decl("dbg_t1", [P, 6 * P], dt.bfloat16, isOutput=True)
        dbg_stats = decl("dbg_stats", [P, 6], dt.float32, isOutput=True)
        dbg_table2 = decl("dbg_table2", [NP, D], dt.bfloat16, isOutput=True)

    rg = [list(range(NCORES))]

    with tile.TileContext(nc) as tc:
        import contextlib
        with contextlib.ExitStack() as ctx:
            ek = ctx.enter_context
            const = ek(tc.tile_pool(name="const", bufs=1))
            nodeb = ek(tc.tile_pool(name="nodeb", bufs=2))
            edge_ea = ek(tc.tile_pool(name="edge_ea", bufs=2))
            edge_idx = ek(tc.tile_pool(name="edge_idx", bufs=3))
            edge_hr = ek(tc.tile_pool(name="edge_hr", bufs=2))
            edge_msg = ek(tc.tile_pool(name="edge_msg", bufs=2))
            edge_S = ek(tc.tile_pool(name="edge_S", bufs=2))
            small = ek(tc.tile_pool(name="small", bufs=4))
            trp = ek(tc.tile_pool(name="trp", bufs=3))
            ps_mp = ek(tc.tile_pool(name="ps_mp", bufs=3, space="PSUM"))
            ps_conv = ek(tc.tile_pool(name="ps_conv", bufs=2, space="PSUM"))
            ps_node = ek(tc.tile_pool(name="ps_node", bufs=1, space="PSUM"))
            ps_tr = ek(tc.tile_pool(name="ps_tr", bufs=1, space="PSUM"))
            dram = ek(tc.tile_pool(name="dram", bufs=2, space="DRAM"))

            # ---- load constants ----
            def ld(pool, shape, dty, src, name):
                t = pool.tile(shape, dty, name=name)
                nc.sync.dma_start(out=t[:], in_=src[...])
                return t

            xT_t = ld(const, [P, NSH], dt.bfloat16, xT, 'xT_t')
            dinv_t = ld(const, [P, NSH], dt.bfloat16, dinv_d, 'dinv_d_t')
            iota_t = ld(const, [P, P], dt.uint8, iota_d, 'iota_d_t')
            ident_t = ld(const, [P, P], dt.bfloat16, ident_d, 'ident_d_t')
            colrel_t = ld(const, [P, TC], dt.uint8, colrel_d, 'colrel_d_t')
            W_t = [ld(const, [D, D], dt.bfloat16, W_d[i], f'W_t{i}') for i in range(3)]
            Wf_t = [ld(const, [D, D], dt.float32, Wf_d[i], f'Wf_t{i}') for i in range(2)]
            Wl_t = ld(const, [D, D], dt.bfloat16, Wl_d, 'Wl_d_t')
            Wlf_t = ld(const, [D, D], dt.float32, Wlf_d, 'Wlf_d_t')
            We_t = [ld(const, [DE, D], dt.bfloat16, We_d[i], f'We_t{i}') for i in range(3)]
            bcol1_t = ld(const, [D, 1], dt.float32, bcol1_d, 'bcol1_d_t')
            brow_t = [ld(const, [1, D], dt.float32, brow_d[i], f'brow_t{i}') for i in range(2)]
            blrow_t = ld(const, [1, D], dt.float32, blrow_d, 'blrow_d_t')
            g_t = [ld(const, [D, 1], dt.float32, g_d[i], f'g_t{i}') for i in range(3)]
            bt_t = [ld(const, [D, 1], dt.float32, bt_d[i], f'bt_t{i}') for i in range(3)]

            t_T = [const.tile([P, NSH], dt.bfloat16, name=f't_T{i}') for i in range(2)]
            eps_t = const.tile([P, 1], dt.float32, name='eps_t')
            nc.vector.memset(eps_t[:], EPS)

            m_lo, m_hi = sched["m_lo"], sched["m_hi"]
            chunk_off, s_off, s_lo = sched["chunk_off"], sched["s_off"], sched["s_lo"]
            qctr = [0]  # round-robin SWDGE queue for dma_gather

            col_chunks = [(o, min(512, NSH - o)) for o in range(0, NSH, 512)]

            def node_linear(rhs_t, Wp_t, bp_t, hout_t):
                """hout = dinv * (W'.T @ rhs + b')  (feature-major)."""
                for (o, w) in col_chunks:
                    pp = ps_node.tile([P, w], dt.float32, space="PSUM",
                                         padded_shape=[P, 512])
                    nc.tensor.matmul(out=pp[:], lhsT=Wp_t[:], rhs=rhs_t[:, o:o + w],
                                     start=True, stop=True)
                    tmp = trp.tile([P, w], dt.bfloat16)
                    nc.scalar.activation(out=tmp[:], in_=pp[:], func=AF.Identity,
                                         bias=bp_t[:])
                    nc.vector.tensor_tensor(out=hout_t[:, o:o + w], in0=tmp[:],
                                            in1=dinv_t[:, o:o + w], op=ALU.mult)

            for k in range(3):  # layers 1..3
                # ---- fold previous BN (k>=1) into this layer's weights ----
                if k == 0:
                    Wp_t, bp_t = W_t[0], bcol1_t
                    rhs_t = xT_t
                else:
                    a_t, c_t = bn_fold  # from previous layer epilogue
                    Wp_t = small.tile([D, D], dt.bfloat16)
                    nc.scalar.activation(out=Wp_t[:], in_=W_t[k][:], func=AF.Identity,
                                         scale=a_t[:])
                    pb = ps_tr.tile([1, D], dt.float32, space="PSUM",
                                    padded_shape=[1, 512])
                    nc.tensor.matmul(out=pb[:], lhsT=c_t[:], rhs=Wf_t[k - 1][:],
                                     start=True, stop=True)
                    bprow = small.tile([1, D], dt.float32)
                    nc.vector.tensor_tensor(out=bprow[:], in0=pb[:], in1=brow_t[k - 1][:],
                                            op=ALU.add)
                    bp_bounce = dram.tile([1, D], dt.float32, name='bp_bounce')
                    nc.sync.dma_start(out=bp_bounce[:], in_=bprow[:])
                    bp_t = small.tile([D, 1], dt.float32)
                    nc.sync.dma_start(out=bp_t[:], in_=bp_bounce[0, :, None])
                    rhs_t = t_T[(k - 1) % 2]

                # ---- node linear + dinv scale ----
                hlin_t = nodeb.tile([P, NSH], dt.bfloat16)
                node_linear(rhs_t, Wp_t, bp_t, hlin_t)

                # ---- transpose tiles to node-major, write shard, AllGather ----
                shard = dram.tile([NSH, D], dt.bfloat16)
                table = dram.tile([NP, D], dt.bfloat16)
                for t in range(BPC):
                    ptr = ps_tr.tile([P, P], dt.bfloat16, space="PSUM",
                                     padded_shape=[P, 1024])
                    nc.tensor.transpose(out=ptr[:], in_=hlin_t[:, t * P:(t + 1) * P],
                                        identity=ident_t[:])
                    sb = trp.tile([P, P], dt.bfloat16)
                    nc.scalar.activation(out=sb[:], in_=ptr[:], func=AF.Copy)
                    nc.sync.dma_start(out=shard[t * P:(t + 1) * P, :], in_=sb[:])
                if not SKIP_CC:
                    nc.gpsimd.collective_compute(
                        "AllGather", ALU.bypass, replica_groups=rg,
                        ins=[shard[:].opt()], outs=[table[:].opt()],
                    )
                else:
                    nc.sync.dma_start(out=table[:NSH, :], in_=shard[:, :])
                if DBG and k == 0:
                    nc.sync.dma_start(out=dbg_table[:, :], in_=table[:, :])
                if DBG and k == 1:
                    nc.sync.dma_start(out=dbg_table2[:, :], in_=table[:, :])

                # ---- edge phase over 49 dest blocks ----
                sums_t = small.tile([P, BPC], dt.float32)
                sqs_t = small.tile([P, BPC], dt.float32)
                tnew = t_T[k % 2]
                for b in range(BPC):
                    m = m_lo[b] + m_hi[b]
                    co = chunk_off[b]
                    ea_t = edge_ea.tile([DE, m, P], dt.bfloat16)
                    if not SKIP_EA:
                        nc.sync.dma_start(out=ea_t[:], in_=eaT_d[:, co:co + m, :])
                    else:
                        nc.vector.memset(ea_t[:, :1, :], 0.0)
                    stot_b = s_lo[b] + m_hi[b] * 8
                    idx_t = edge_idx.tile([P, stot_b], dt.int16, name=f'idxb')
                    nc.sync.dma_start(out=idx_t[:], in_=idx_d[:, s_off[b]:s_off[b] + stot_b])
                    hr_t = edge_hr.tile([P, m, D], dt.bfloat16)
                    # dma_gather's Q7 ucode handles at most 1024 indices
                    # per call -> split into <=8-chunk (1024-edge) pieces.
                    # queue q executes on Q7 core pair (2q, 2q+1); cycling
                    # queues lets 4 gathers' desc-gen run concurrently.
                    if not SKIP_GATHER:
                        for mh, tab, mbase, sbase in (
                            (m_lo[b], table[:VHALF, :], 0, 0),
                            (m_hi[b], table[VHALF:, :], m_lo[b], s_lo[b]),
                        ):
                            for pc in range(0, mh, 8):
                                pw = min(8, mh - pc)
                                nc.gpsimd.dma_gather(
                                    out_ap=hr_t[:, mbase + pc:mbase + pc + pw, :],
                                    in_ap=tab,
                                    idxs_ap=idx_t[:, sbase + pc * 8:sbase + (pc + pw) * 8],
                                    num_idxs=pw * P, num_idxs_reg=pw * P, elem_size=D,
                                    queue_num=qctr[0],
                                )
                                qctr[0] = (qctr[0] + 1) % NQ
                    else:
                        nc.vector.memset(hr_t[:, :1, :], 0.25)
                    if DBG and k == 0 and b == 0:
                        dbg_hr_sb = trp.tile([P, 4 * D], dt.bfloat16, name='dbg_hr_sb')
                        nc.vector.tensor_copy(out=dbg_hr_sb[:], in_=hr_t[:, :4, :].rearrange("p m d -> p (m d)"))
                        nc.sync.dma_start(out=dbg_hr[:, :], in_=dbg_hr_sb[:])
                    # S indicator [P, m, P]
                    S_t = edge_S.tile([P, m, P], S_DTYPE)
                    iota_b = bass.AP(tensor=iota_t.tensor, offset=iota_t[:].offset,
                                     ap=[iota_t[:].ap[0], [0, m], iota_t[:].ap[1]])
                    cr = colrel_t[:, co:co + m]
                    cr_b = bass.AP(tensor=colrel_t.tensor, offset=cr.offset,
                                   ap=[cr.ap[0], cr.ap[1], [0, P]])
                    nc.vector.tensor_tensor(out=S_t[:], in0=iota_b, in1=cr_b,
                                            op=ALU.is_equal)
                    # messages
                    msg_t = edge_msg.tile([P, m, D], dt.bfloat16)
                    if SKIP_EDGE:
                        nc.vector.memset(msg_t[:, :1, :], 0.1)
                    j = 0 if not SKIP_EDGE else m
                    while j < m:
                        jw = min(4, m - j)
                        mp = ps_mp.tile([P, 4, D], dt.float32, space="PSUM")
                        # start=True zeroes the whole 2KB bank, so the
                        # full-tile identity-add must come first.
                        nc.tensor.matmul(
                            out=mp[:, :jw, :].rearrange("p j d -> p (j d)"),
                            lhsT=ident_t[:],
                            rhs=hr_t[:, j:j + jw, :].rearrange("p j d -> p (j d)"),
                            start=True, stop=False, skip_group_check=True)
                        for jj in range(jw):
                            nc.tensor.matmul(out=mp[:, jj, :],
                                             lhsT=ea_t[:, j + jj, :], rhs=We_t[k][:],
                                             start=False, stop=(jj == jw - 1),
                                             skip_group_check=True)
                        nc.scalar.activation(
                            out=msg_t[:, j:j + jw, :].rearrange("p j d -> p (j d)"),
                            in_=mp[:, :jw, :].rearrange("p j d -> p (j d)"), func=AF.Relu)
                        j += jw
                    if DBG and k == 0 and b == 0:
                        dbg_msg_sb = trp.tile([P, 4 * D], dt.bfloat16, name='dbg_msg_sb')
                        nc.vector.tensor_copy(out=dbg_msg_sb[:], in_=msg_t[:, :4, :].rearrange("p m d -> p (m d)"))
                        nc.sync.dma_start(out=dbg_msg[:, :], in_=dbg_msg_sb[:])
                    # scatter into conv accumulator (feature-major out)
                    cp = ps_conv.tile([P, P], dt.float32, space="PSUM",
                                      padded_shape=[P, 512])
                    for j in range(m if not SKIP_EDGE else 1):
                        nc.tensor.matmul(out=cp[:], lhsT=msg_t[:, j, :],
                                         rhs=S_t[:, j, :],
                                         start=(j == 0), stop=(j == m - 1))
                    # epilogue: dinv scale, relu -> t, stats
                    sl = slice(b * P, (b + 1) * P)
                    pre = trp.tile([P, P], dt.float32)
                    nc.vector.tensor_tensor(out=pre[:], in0=cp[:],
                                            in1=dinv_t[:, sl], op=ALU.mult)
                    nc.scalar.activation(out=tnew[:, sl], in_=pre[:], func=AF.Relu,
                                         accum_out=sums_t[:, b:b + 1])
                    sq_scr = trp.tile([P, P], dt.bfloat16)
                    nc.scalar.activation(out=sq_scr[:], in_=tnew[:, sl], func=AF.Square,
                                         accum_out=sqs_t[:, b:b + 1])

                # ---- BN stats + fold coefficients ----
                st = small.tile([P, 2], dt.float32)
                nc.vector.tensor_reduce(out=st[:, 0:1], in_=sums_t[:],
                                        axis=mybir.AxisListType.X, op=ALU.add)
                nc.vector.tensor_reduce(out=st[:, 1:2], in_=sqs_t[:],
                                        axis=mybir.AxisListType.X, op=ALU.add)
                st_in = dram.tile([P, 2], dt.float32)
                st_out = dram.tile([P, 2], dt.float32)
                nc.sync.dma_start(out=st_in[:], in_=st[:])
                if not SKIP_CC:
                    nc.gpsimd.collective_compute(
                        "AllReduce", ALU.add, replica_groups=rg,
                        ins=[st_in[:].opt()], outs=[st_out[:].opt()],
                    )
                else:
                    nc.sync.dma_start(out=st_out[:, :], in_=st_in[:, :])
                stg = small.tile([P, 2], dt.float32)
                nc.sync.dma_start(out=stg[:], in_=st_out[:])
                mu = small.tile([P, 1], dt.float32)
                nc.vector.tensor_scalar(out=mu[:], in0=stg[:, 0:1], scalar1=1.0 / N,
                                        scalar2=None, op0=ALU.mult)
                ex2 = small.tile([P, 1], dt.float32)
                nc.vector.tensor_scalar(out=ex2[:], in0=stg[:, 1:2], scalar1=1.0 / N,
                                        scalar2=None, op0=ALU.mult)
                var = small.tile([P, 1], dt.float32)
                nc.vector.tensor_tensor(out=var[:], in0=mu[:], in1=mu[:], op=ALU.mult)
                nc.vector.tensor_tensor(out=var[:], in0=ex2[:], in1=var[:],
                                        op=ALU.subtract)
                sd = small.tile([P, 1], dt.float32)
                nc.scalar.activation(out=sd[:], in_=var[:], func=AF.Sqrt, bias=eps_t[:])
                rs = small.tile([P, 1], dt.float32)
                nc.vector.reciprocal(out=rs[:], in_=sd[:])
                a_t = small.tile([P, 1], dt.float32)
                nc.vector.tensor_tensor(out=a_t[:], in0=rs[:], in1=g_t[k][:],
                                        op=ALU.mult)
                c_t = small.tile([P, 1], dt.float32)
                nc.vector.tensor_tensor(out=c_t[:], in0=mu[:], in1=a_t[:], op=ALU.mult)
                nc.vector.tensor_tensor(out=c_t[:], in0=bt_t[k][:], in1=c_t[:],
                                        op=ALU.subtract)
                bn_fold = (a_t, c_t)
                if DBG and k == 0:
                    nt = min(6 * P, NSH)
                    dbg_t1_sb = trp.tile([P, 6 * P], dt.bfloat16, name='dbg_t1_sb')
                    nc.vector.tensor_copy(out=dbg_t1_sb[:, :nt], in_=tnew[:, :nt])
                    nc.sync.dma_start(out=dbg_t1[:, :], in_=dbg_t1_sb[:])
                    dbg_st_sb = trp.tile([P, 6], dt.float32, name='dbg_st_sb')
                    for ii, tt in enumerate((mu, ex2, var, sd, a_t, c_t)):
                        nc.vector.tensor_copy(out=dbg_st_sb[:, ii:ii+1], in_=tt[:])
                    nc.sync.dma_start(out=dbg_stats[:, :], in_=dbg_st_sb[:])

            # ---- final linear: out^T = Wl'.T @ t3 + bl' ----
            a_t, c_t = bn_fold
            Wlp = small.tile([D, D], dt.bfloat16)
            nc.scalar.activation(out=Wlp[:], in_=Wl_t[:], func=AF.Identity, scale=a_t[:])
            pb = ps_tr.tile([1, D], dt.float32, space="PSUM",
                            padded_shape=[1, 512])
            nc.tensor.matmul(out=pb[:], lhsT=c_t[:], rhs=Wlf_t[:], start=True, stop=True)
            blp_row = small.tile([1, D], dt.float32)
            nc.vector.tensor_tensor(out=blp_row[:], in0=pb[:], in1=blrow_t[:], op=ALU.add)
            blp_bounce = dram.tile([1, D], dt.float32, name='blp_bounce')
            nc.sync.dma_start(out=blp_bounce[:], in_=blp_row[:])
            blp = small.tile([D, 1], dt.float32)
            nc.sync.dma_start(out=blp[:], in_=blp_bounce[0, :, None])
            for (o, w) in col_chunks:
                pp = ps_node.tile([P, w], dt.float32, space="PSUM",
                                     padded_shape=[P, 512])
                nc.tensor.matmul(out=pp[:], lhsT=Wlp[:], rhs=t_T[0][:, o:o + w],
                                 start=True, stop=True)
                ot = trp.tile([P, w], dt.float32)
                nc.scalar.activation(out=ot[:], in_=pp[:], func=AF.Identity, bias=blp[:])
                nc.sync.dma_start(out=outT[:, o:o + w], in_=ot[:])

    nc.finalize()
    return nc


# ---------------- public entry point ----------------

_CACHE = {}
LAST_EXEC_NS = None


def _make_in_maps(inputs, per_core, dinv):
    x = np.asarray(inputs["x"], np.float32)

    dinv_pad = np.zeros(NP, np.float32)
    dinv_pad[:N] = dinv
    xT_full = np.zeros((P, NP), np.float32)
    xT_full[:, :N] = x.T

    Ws = {k: np.asarray(inputs[k], np.float32) for k in
          ("W1", "W2", "W3", "Wl", "We1", "We2", "We3")}
    bt_tot = {k: np.asarray(inputs[f"b{k}"], np.float32) +
                 np.asarray(inputs[f"be{k}"], np.float32) for k in (1, 2, 3)}

    in_maps = []
    for c in range(NCORES):
        sl = slice(c * NSH, (c + 1) * NSH)
        im = dict(per_core[c])
        im["xT"] = _bf16(xT_full[:, sl])
        im["dinvt"] = _bf16(np.tile(dinv_pad[sl][None, :], (P, 1)))
        im["iota_u8"] = np.tile(np.arange(P, dtype=np.uint8)[None, :], (P, 1))
        im["ident"] = _bf16(np.eye(P))
        for i, k in enumerate((1, 2, 3)):
            im[f"W{k}"] = _bf16(Ws[f"W{k}"])
            im[f"We{k}"] = _bf16(Ws[f"We{k}"])
            im[f"g{k}"] = np.asarray(inputs[f"g{k}"], np.float32).reshape(D, 1)
            im[f"bt{k}"] = np.asarray(inputs[f"bt{k}"], np.float32).reshape(D, 1)
        im["Wf2"] = Ws["W2"]
        im["Wf3"] = Ws["W3"]
        im["Wlin"] = _bf16(Ws["Wl"])
        im["Wlinf"] = Ws["Wl"]
        im["bcol1"] = bt_tot[1].reshape(D, 1)
        im["brow2"] = bt_tot[2].reshape(1, D)
        im["brow3"] = bt_tot[3].reshape(1, D)
        im["blrow"] = np.asarray(inputs["bl"], np.float32).reshape(1, D)
        in_maps.append(im)
    return in_maps


def kernel(**inputs):
    edge_attr = np.asarray(inputs["edge_attr"], np.float32)
    edge_index = np.asarray(inputs["edge_index"])

    per_core, sched, dinv = _preprocess(edge_index, edge_attr)
    in_maps = _make_in_maps(inputs, per_core, dinv)

    key = ("k", sched["tot_chunks"], sched["tot_s"],
           tuple(sched["m_lo"]), tuple(sched["m_hi"]))
    if key not in _CACHE:
        _CACHE[key] = _build(sched)
    nc = _CACHE[key]

    import os
    trace = os.environ.get("KPROF") == "1"
    r = run_bass_kernel_spmd(nc, in_maps, core_ids=list(range(NCORES)), trace=trace)
    if trace:
        print(f"HW exec time: {r.exec_time_ns} ns", flush=True)
        global LAST_EXEC_NS
        LAST_EXEC_NS = r.exec_time_ns
        try:
            insts = r.instructions_and_trace[0] if r.instructions_and_trace else []
            import pickle
            rows = [
                dict(ts=i.timestamp, dur=i.duration, engine=str(i.engine),
                     name=i.name, label=i.label, wait=i.evt_wait_time,
                     bir=str(i.bir_str)[:200], src=f"{i.source_file}:{i.source_line}")
                for i in insts
            ]
            with open("/tmp/kprof_insts.pkl", "wb") as f:
                pickle.dump(rows, f)
            print(f"KPROF: dumped {len(rows)} insts; trace="
                  f"{r.instructions_and_trace[1] if r.instructions_and_trace else None}",
                  flush=True)
        except Exception as e:
            print(f"KPROF dump failed: {e}", flush=True)
    res = r.results
    outT = np.concatenate([res[c]["outT"] for c in range(NCORES)], axis=1)  # [128, NP]
    return np.ascontiguousarray(outT.T[:N]).astype(np.float32)



# revision 16
# speedup vs baseline: 2.3824x; 1.0349x over previous
"""GCN (3x GCNConv + BN + final linear) on 8 TRN2 NeuronCores.

Strategy (see test.py for the harness):
- Pad N=50000 -> NP=50176 = 392 blocks of 128 nodes. Core c owns 49
  blocks (6272 nodes) and all edges whose destination (col) lies in them.
- The GCN norm dinv[row]*dinv[col] is factorized: dinv[row] is folded into
  edge_attr (host) and into the gather table rows (device); dinv[col] is
  applied to the aggregated block output (device).
- BatchNorm+bias are affine per-feature and are folded into the next
  layer's weights on device, so the edge phase is just
  msg = relu(table[row] + ea' @ We); out[col] += msg,
  computed as dense matmuls: per 128-edge chunk an indicator one-hot
  matrix S (built on DVE from uint8 compares) scatters messages into a
  PSUM accumulator per destination block; gathers use the hardware
  dma_gather (int16 indices, table split in two 25088-row halves).
- Per layer: node linear (feature-major) -> AllGather bf16 table ->
  edge phase -> BN stats AllReduce -> fold affine into next weights.
"""

import sys

sys.path.insert(0, "/opt/trn_rl_repo")

import numpy as np
import ml_dtypes

import concourse.bass as bass
import concourse.tile as tile
from concourse import bacc, mybir
from concourse.bass_utils import run_bass_kernel_spmd

# ---------------- constants ----------------
NCORES = 8
D = 128
DE = 16
EPS = 1e-5
P = 128


def configure(n):
    """Set the node count; everything else derives from it."""
    global N, BLOCKS, NP, BPC, NSH, VHALF
    N = n
    BLOCKS = ((N + P - 1) // P + NCORES - 1) // NCORES * NCORES
    NP = BLOCKS * P
    BPC = BLOCKS // NCORES
    NSH = BPC * P
    VHALF = NP // 2


configure(50000)
E = 1_600_000

dt = mybir.dt
AF = mybir.ActivationFunctionType
ALU = mybir.AluOpType

S_DTYPE = dt.bfloat16        # indicator matrix dtype


def _bf16(a):
    return np.asarray(a, dtype=np.float32).astype(ml_dtypes.bfloat16)


# ---------------- host-side edge preprocessing ----------------

def _preprocess(edge_index, edge_attr):
    """Sort/pad edges per (core, dest-block); build packed device arrays.

    Returns dict of per-core numpy arrays + the uniform chunk schedule.
    """
    row = np.asarray(edge_index[0], dtype=np.int64)
    col = np.asarray(edge_index[1], dtype=np.int64)
    deg = np.bincount(row, minlength=N).astype(np.float32) + 1.0
    dinv = deg ** -0.5                                  # [N]
    ea_s = np.asarray(edge_attr, np.float32) * dinv[row][:, None]   # [E,16]

    blk = col // P                                      # dest block of each edge
    order = np.argsort(blk, kind="stable")
    row_s, col_s, blk_s = row[order], col[order], blk[order]
    ea_sorted = ea_s[order]
    # boundaries per block
    starts = np.searchsorted(blk_s, np.arange(BLOCKS))
    ends = np.searchsorted(blk_s, np.arange(BLOCKS), side="right")

    # per (core, local block): split lo/hi rows, sizes
    lists = [[None] * BPC for _ in range(NCORES)]
    n_lo = np.zeros((NCORES, BPC), np.int64)
    n_hi = np.zeros((NCORES, BPC), np.int64)
    for g in range(BLOCKS):
        c, b = divmod(g, BPC)
        s, e = starts[g], ends[g]
        r = row_s[s:e]
        lo_mask = r < VHALF
        lo_idx = np.nonzero(lo_mask)[0]
        hi_idx = np.nonzero(~lo_mask)[0]
        lists[c][b] = (s, lo_idx, hi_idx)
        n_lo[c, b] = len(lo_idx)
        n_hi[c, b] = len(hi_idx)

    # uniform chunk counts across cores
    m_lo = np.maximum(1, (n_lo.max(axis=0) + P - 1) // P).astype(int)   # [BPC]
    m_hi = np.maximum(1, (n_hi.max(axis=0) + P - 1) // P).astype(int)
    m_tot = m_lo + m_hi
    tot_chunks = int(m_tot.sum())
    chunk_off = np.zeros(BPC, int)
    chunk_off[1:] = np.cumsum(m_tot)[:-1]
    # 4-chunk groups for the batched (block-diag) edge-feature matmul
    n_grp = (m_tot + 3) // 4
    tot_grp = int(n_grp.sum())
    grp_off = np.zeros(BPC, int)
    grp_off[1:] = np.cumsum(n_grp)[:-1]
    # idx columns (16-wrapped) offsets, in units of int16 columns
    s_lo = m_lo * 8
    s_hi = m_hi * 8
    s_tot = s_lo + s_hi
    tot_s = int(s_tot.sum())
    s_off = np.zeros(BPC, int)
    s_off[1:] = np.cumsum(s_tot)[:-1]

    per_core = []
    for c in range(NCORES):
        eaT = np.zeros((DE, tot_chunks, P), np.float32)
        colrel = np.full((P, tot_chunks), 255, np.uint8)
        idx16 = np.zeros((16, tot_s), np.int16)
        for b in range(BPC):
            s, lo_idx, hi_idx = lists[c][b]
            co = chunk_off[b]
            for half, sub, m_half, half_chunk_base, base in (
                (0, lo_idx, m_lo[b], 0, 0),
                (1, hi_idx, m_hi[b], m_lo[b], VHALF),
            ):
                g_sz = int(m_half) * P
                rows_h = np.zeros(g_sz, np.int64)        # pad idx -> 0
                rows_h[: len(sub)] = row_s[s + sub] - base
                cols_h = np.full(g_sz, 255, np.int64)    # pad col -> 255
                cols_h[: len(sub)] = col_s[s + sub] - (c * BPC + b) * P
                ea_h = np.zeros((g_sz, DE), np.float32)
                ea_h[: len(sub)] = ea_sorted[s + sub]
                ii = np.arange(g_sz)
                pp, jj = ii % P, ii // P
                eaT[:, co + half_chunk_base + jj, pp] = ea_h.T
                colrel[pp, co + half_chunk_base + jj] = cols_h
                # 16-wrapped idx at column offset
                so = s_off[b] + (0 if half == 0 else s_lo[b])
                idx16[ii % 16, so + ii // 16] = rows_h
        # repack eaT [16, chunk, 128] -> [64, group, 128]: group g holds
        # chunks 4g..4g+3 of its block stacked along the partition axis, so
        # one matmul against a block-diagonal We computes e for 4 chunks.
        eaT_pack = np.zeros((4 * DE, tot_grp, P), np.float32)
        for b in range(BPC):
            m = int(m_tot[b])
            for j in range(m):
                eaT_pack[16 * (j % 4):16 * (j % 4) + 16, grp_off[b] + j // 4, :] = \
                    eaT[:, chunk_off[b] + j, :]
        per_core.append(
            dict(
                eaT=_bf16(eaT_pack),
                colrel=colrel,
                idx16=np.tile(idx16, (8, 1)),            # replicate to 128 partitions
            )
        )

    sched = dict(
        m_lo=[int(v) for v in m_lo], m_hi=[int(v) for v in m_hi],
        chunk_off=[int(v) for v in chunk_off], s_off=[int(v) for v in s_off],
        s_lo=[int(v) for v in s_lo],
        n_grp=[int(v) for v in n_grp], grp_off=[int(v) for v in grp_off],
        tot_chunks=tot_chunks, tot_s=tot_s, tot_grp=tot_grp,
    )
    return per_core, sched, dinv


# ---------------- device program ----------------

NQ = 4  # SWDGE queues: dma_gather queue q runs on Q7 core pair (2q, 2q+1),
        # so round-robin over 4 queues runs 4 gathers concurrently.


def _build(sched):
    nc = bacc.Bacc(None, target_bir_lowering=False, debug=False,
                   num_swdge_queues=NQ)
    TC, TS = sched["tot_chunks"], sched["tot_s"]
    TG = sched["tot_grp"]

    # ---- external inputs (per-core shapes) ----
    decl = nc.declare_dram_parameter
    xT = decl("xT", [P, NSH], dt.bfloat16, isOutput=False)
    eaT_d = decl("eaT", [4 * DE, TG, P], dt.bfloat16, isOutput=False)
    colrel_d = decl("colrel", [P, TC], dt.uint8, isOutput=False)
    idx_d = decl("idx16", [P, TS], dt.int16, isOutput=False)
    dinv_d = decl("dinvt", [P, NSH], dt.bfloat16, isOutput=False)
    iota_d = decl("iota_u8", [P, P], dt.uint8, isOutput=False)
    ident_d = decl("ident", [P, P], dt.bfloat16, isOutput=False)
    W_d = [decl(f"W{k}", [D, D], dt.bfloat16, isOutput=False) for k in (1, 2, 3)]
    Wf_d = [decl(f"Wf{k}", [D, D], dt.float32, isOutput=False) for k in (2, 3)]
    Wl_d = decl("Wlin", [D, D], dt.bfloat16, isOutput=False)
    Wlf_d = decl("Wlinf", [D, D], dt.float32, isOutput=False)
    We_d = [decl(f"We{k}", [4 * DE, 4 * D], dt.bfloat16, isOutput=False)
            for k in (1, 2, 3)]  # block-diag: 4 copies of We on the diagonal
    # b_tot[k] = b_k + be_k as column [128,1]; rows [1,128] for fold matmuls
    bcol1_d = decl("bcol1", [D, 1], dt.float32, isOutput=False)
    brow_d = [decl(f"brow{k}", [1, D], dt.float32, isOutput=False) for k in (2, 3)]
    blrow_d = decl("blrow", [1, D], dt.float32, isOutput=False)
    g_d = [decl(f"g{k}", [D, 1], dt.float32, isOutput=False) for k in (1, 2, 3)]
    bt_d = [decl(f"bt{k}", [D, 1], dt.float32, isOutput=False) for k in (1, 2, 3)]
    outT = decl("outT", [P, NSH], dt.float32, isOutput=True)
    import os
    DBG = os.environ.get("KDBG") == "1"
    SKIP_GATHER = os.environ.get("KSKIP_GATHER") == "1"
    SKIP_CC = os.environ.get("KSKIP_CC") == "1"
    SKIP_EDGE = os.environ.get("KSKIP_EDGE") == "1"
    SKIP_EA = os.environ.get("KSKIP_EA") == "1"
    if DBG:
        dbg_table = decl("dbg_table", [NP, D], dt.bfloat16, isOutput=True)
        dbg_hr = decl("dbg_hr", [P, 4 * D], dt.bfloat16, isOutput=True)
        dbg_msg = decl("dbg_msg", [P, 4 * D], dt.bfloat16, isOutput=True)
        dbg_t1 = decl("dbg_t1", [P, 6 * P], dt.bfloat16, isOutput=True)
        dbg_stats = decl("dbg_stats", [P, 6], dt.float32, isOutput=True)
        dbg_table2 = decl("dbg_table2", [NP, D], dt.bfloat16, isOutput=True)

    rg = [list(range(NCORES))]

    with tile.TileContext(nc) as tc:
        import contextlib
        with contextlib.ExitStack() as ctx:
            ek = ctx.enter_context
            const = ek(tc.tile_pool(name="const", bufs=1))
            nodeb = ek(tc.tile_pool(name="nodeb", bufs=2))
            edge_ea = ek(tc.tile_pool(name="edge_ea", bufs=3))
            edge_idx = ek(tc.tile_pool(name="edge_idx", bufs=4))
            edge_hr = ek(tc.tile_pool(name="edge_hr", bufs=3))
            edge_msg = ek(tc.tile_pool(name="edge_msg", bufs=2))
            edge_S = ek(tc.tile_pool(name="edge_S", bufs=3))
            small = ek(tc.tile_pool(name="small", bufs=4))
            trp = ek(tc.tile_pool(name="trp", bufs=3))
            ps_mp = ek(tc.tile_pool(name="ps_mp", bufs=3, space="PSUM"))
            ps_conv = ek(tc.tile_pool(name="ps_conv", bufs=2, space="PSUM"))
            ps_node = ek(tc.tile_pool(name="ps_node", bufs=1, space="PSUM"))
            ps_tr = ek(tc.tile_pool(name="ps_tr", bufs=1, space="PSUM"))
            dram = ek(tc.tile_pool(name="dram", bufs=2, space="DRAM"))

            # ---- load constants ----
            def ld(pool, shape, dty, src, name):
                t = pool.tile(shape, dty, name=name)
                nc.sync.dma_start(out=t[:], in_=src[...])
                return t

            xT_t = ld(const, [P, NSH], dt.bfloat16, xT, 'xT_t')
            dinv_t = ld(const, [P, NSH], dt.bfloat16, dinv_d, 'dinv_d_t')
            iota_t = ld(const, [P, P], dt.uint8, iota_d, 'iota_d_t')
            ident_t = ld(const, [P, P], dt.bfloat16, ident_d, 'ident_d_t')
            colrel_t = ld(const, [P, TC], dt.uint8, colrel_d, 'colrel_d_t')
            W_t = [ld(const, [D, D], dt.bfloat16, W_d[i], f'W_t{i}') for i in range(3)]
            Wf_t = [ld(const, [D, D], dt.float32, Wf_d[i], f'Wf_t{i}') for i in range(2)]
            Wl_t = ld(const, [D, D], dt.bfloat16, Wl_d, 'Wl_d_t')
            Wlf_t = ld(const, [D, D], dt.float32, Wlf_d, 'Wlf_d_t')
            We_t = [ld(const, [4 * DE, 4 * D], dt.bfloat16, We_d[i], f'We_t{i}')
                    for i in range(3)]
            bcol1_t = ld(const, [D, 1], dt.float32, bcol1_d, 'bcol1_d_t')
            brow_t = [ld(const, [1, D], dt.float32, brow_d[i], f'brow_t{i}') for i in range(2)]
            blrow_t = ld(const, [1, D], dt.float32, blrow_d, 'blrow_d_t')
            g_t = [ld(const, [D, 1], dt.float32, g_d[i], f'g_t{i}') for i in range(3)]
            bt_t = [ld(const, [D, 1], dt.float32, bt_d[i], f'bt_t{i}') for i in range(3)]

            t_T = [const.tile([P, NSH], dt.bfloat16, name=f't_T{i}') for i in range(2)]
            eps_t = const.tile([P, 1], dt.float32, name='eps_t')
            nc.vector.memset(eps_t[:], EPS)

            m_lo, m_hi = sched["m_lo"], sched["m_hi"]
            chunk_off, s_off, s_lo = sched["chunk_off"], sched["s_off"], sched["s_lo"]
            n_grp, grp_off = sched["n_grp"], sched["grp_off"]
            qctr = [0]  # round-robin SWDGE queue for dma_gather

            col_chunks = [(o, min(512, NSH - o)) for o in range(0, NSH, 512)]

            def node_linear(rhs_t, Wp_t, bp_t, hout_t):
                """hout = dinv * (W'.T @ rhs + b')  (feature-major)."""
                for (o, w) in col_chunks:
                    pp = ps_node.tile([P, w], dt.float32, space="PSUM",
                                         padded_shape=[P, 512])
                    nc.tensor.matmul(out=pp[:], lhsT=Wp_t[:], rhs=rhs_t[:, o:o + w],
                                     start=True, stop=True)
                    tmp = trp.tile([P, w], dt.bfloat16)
                    nc.scalar.activation(out=tmp[:], in_=pp[:], func=AF.Identity,
                                         bias=bp_t[:])
                    nc.vector.tensor_tensor(out=hout_t[:, o:o + w], in0=tmp[:],
                                            in1=dinv_t[:, o:o + w], op=ALU.mult)

            for k in range(3):  # layers 1..3
                # ---- fold previous BN (k>=1) into this layer's weights ----
                if k == 0:
                    Wp_t, bp_t = W_t[0], bcol1_t
                    rhs_t = xT_t
                else:
                    a_t, c_t = bn_fold  # from previous layer epilogue
                    Wp_t = small.tile([D, D], dt.bfloat16)
                    nc.scalar.activation(out=Wp_t[:], in_=W_t[k][:], func=AF.Identity,
                                         scale=a_t[:])
                    pb = ps_tr.tile([1, D], dt.float32, space="PSUM",
                                    padded_shape=[1, 512])
                    nc.tensor.matmul(out=pb[:], lhsT=c_t[:], rhs=Wf_t[k - 1][:],
                                     start=True, stop=True)
                    bprow = small.tile([1, D], dt.float32)
                    nc.vector.tensor_tensor(out=bprow[:], in0=pb[:], in1=brow_t[k - 1][:],
                                            op=ALU.add)
                    bp_bounce = dram.tile([1, D], dt.float32, name='bp_bounce')
                    nc.sync.dma_start(out=bp_bounce[:], in_=bprow[:])
                    bp_t = small.tile([D, 1], dt.float32)
                    nc.sync.dma_start(out=bp_t[:], in_=bp_bounce[0, :, None])
                    rhs_t = t_T[(k - 1) % 2]

                # ---- node linear + dinv scale ----
                hlin_t = nodeb.tile([P, NSH], dt.bfloat16)
                node_linear(rhs_t, Wp_t, bp_t, hlin_t)

                # ---- transpose tiles to node-major, write shard, AllGather ----
                shard = dram.tile([NSH, D], dt.bfloat16)
                table = dram.tile([NP, D], dt.bfloat16)
                for t in range(BPC):
                    ptr = ps_tr.tile([P, P], dt.bfloat16, space="PSUM",
                                     padded_shape=[P, 1024])
                    nc.tensor.transpose(out=ptr[:], in_=hlin_t[:, t * P:(t + 1) * P],
                                        identity=ident_t[:])
                    sb = trp.tile([P, P], dt.bfloat16)
                    nc.scalar.activation(out=sb[:], in_=ptr[:], func=AF.Copy)
                    nc.sync.dma_start(out=shard[t * P:(t + 1) * P, :], in_=sb[:])
                if not SKIP_CC:
                    nc.gpsimd.collective_compute(
                        "AllGather", ALU.bypass, replica_groups=rg,
                        ins=[shard[:].opt()], outs=[table[:].opt()],
                    )
                else:
                    nc.sync.dma_start(out=table[:NSH, :], in_=shard[:, :])
                if DBG and k == 0:
                    nc.sync.dma_start(out=dbg_table[:, :], in_=table[:, :])
                if DBG and k == 1:
                    nc.sync.dma_start(out=dbg_table2[:, :], in_=table[:, :])

                # ---- edge phase over 49 dest blocks ----
                sums_t = small.tile([P, BPC], dt.float32)
                sqs_t = small.tile([P, BPC], dt.float32)
                tnew = t_T[k % 2]
                for b in range(BPC):
                    m = m_lo[b] + m_hi[b]
                    co = chunk_off[b]
                    ng, go = n_grp[b], grp_off[b]
                    ea_t = edge_ea.tile([4 * DE, ng, P], dt.bfloat16)
                    if not SKIP_EA:
                        nc.sync.dma_start(out=ea_t[:], in_=eaT_d[:, go:go + ng, :])
                    else:
                        nc.vector.memset(ea_t[:, :1, :], 0.0)
                    stot_b = s_lo[b] + m_hi[b] * 8
                    idx_t = edge_idx.tile([P, stot_b], dt.int16, name=f'idxb')
                    nc.sync.dma_start(out=idx_t[:], in_=idx_d[:, s_off[b]:s_off[b] + stot_b])
                    hr_t = edge_hr.tile([P, m, D], dt.bfloat16)
                    # dma_gather's Q7 ucode handles at most 1024 indices
                    # per call -> split into <=8-chunk (1024-edge) pieces.
                    # queue q executes on Q7 core pair (2q, 2q+1); cycling
                    # queues lets 4 gathers' desc-gen run concurrently.
                    if not SKIP_GATHER:
                        for mh, tab, mbase, sbase in (
                            (m_lo[b], table[:VHALF, :], 0, 0),
                            (m_hi[b], table[VHALF:, :], m_lo[b], s_lo[b]),
                        ):
                            for pc in range(0, mh, 8):
                                pw = min(8, mh - pc)
                                nc.gpsimd.dma_gather(
                                    out_ap=hr_t[:, mbase + pc:mbase + pc + pw, :],
                                    in_ap=tab,
                                    idxs_ap=idx_t[:, sbase + pc * 8:sbase + (pc + pw) * 8],
                                    num_idxs=pw * P, num_idxs_reg=pw * P, elem_size=D,
                                    queue_num=qctr[0],
                                )
                                qctr[0] = (qctr[0] + 1) % NQ
                    else:
                        nc.vector.memset(hr_t[:, :1, :], 0.25)
                    if DBG and k == 0 and b == 0:
                        dbg_hr_sb = trp.tile([P, 4 * D], dt.bfloat16, name='dbg_hr_sb')
                        nc.vector.tensor_copy(out=dbg_hr_sb[:], in_=hr_t[:, :4, :].rearrange("p m d -> p (m d)"))
                        nc.sync.dma_start(out=dbg_hr[:, :], in_=dbg_hr_sb[:])
                    # S indicator [P, m, P]
                    S_t = edge_S.tile([P, m, P], S_DTYPE)
                    iota_b = bass.AP(tensor=iota_t.tensor, offset=iota_t[:].offset,
                                     ap=[iota_t[:].ap[0], [0, m], iota_t[:].ap[1]])
                    cr = colrel_t[:, co:co + m]
                    cr_b = bass.AP(tensor=colrel_t.tensor, offset=cr.offset,
                                   ap=[cr.ap[0], cr.ap[1], [0, P]])
                    nc.vector.tensor_tensor(out=S_t[:], in0=iota_b, in1=cr_b,
                                            op=ALU.is_equal)
                    # messages: per 4-chunk group, identity-inject hr then one
                    # block-diag We matmul computes e for all 4 chunks.
                    msg_t = edge_msg.tile([P, m, D], dt.bfloat16)
                    if SKIP_EDGE:
                        nc.vector.memset(msg_t[:, :1, :], 0.1)
                    for g in range(ng if not SKIP_EDGE else 0):
                        j = 4 * g
                        jw = min(4, m - j)
                        mp = ps_mp.tile([P, 4, D], dt.float32, space="PSUM")
                        # start=True zeroes the whole 2KB bank, so the
                        # full-tile identity-add must come first.
                        nc.tensor.matmul(
                            out=mp[:, :jw, :].rearrange("p j d -> p (j d)"),
                            lhsT=ident_t[:],
                            rhs=hr_t[:, j:j + jw, :].rearrange("p j d -> p (j d)"),
                            start=True, stop=False, skip_group_check=True)
                        nc.tensor.matmul(
                            out=mp[:, :jw, :].rearrange("p j d -> p (j d)"),
                            lhsT=ea_t[:16 * jw, g, :],
                            rhs=We_t[k][:16 * jw, :jw * D],
                            start=False, stop=True, skip_group_check=True)
                        nc.scalar.activation(
                            out=msg_t[:, j:j + jw, :].rearrange("p j d -> p (j d)"),
                            in_=mp[:, :jw, :].rearrange("p j d -> p (j d)"), func=AF.Relu)
                    if DBG and k == 0 and b == 0:
                        dbg_msg_sb = trp.tile([P, 4 * D], dt.bfloat16, name='dbg_msg_sb')
                        nc.vector.tensor_copy(out=dbg_msg_sb[:], in_=msg_t[:, :4, :].rearrange("p m d -> p (m d)"))
                        nc.sync.dma_start(out=dbg_msg[:, :], in_=dbg_msg_sb[:])
                    # scatter into conv accumulator (feature-major out)
                    cp = ps_conv.tile([P, P], dt.float32, space="PSUM",
                                      padded_shape=[P, 512])
                    for j in range(m if not SKIP_EDGE else 1):
                        nc.tensor.matmul(out=cp[:], lhsT=msg_t[:, j, :],
                                         rhs=S_t[:, j, :],
                                         start=(j == 0), stop=(j == m - 1))
                    # epilogue: dinv scale, relu -> t, stats
                    sl = slice(b * P, (b + 1) * P)
                    pre = trp.tile([P, P], dt.float32)
                    nc.vector.tensor_tensor(out=pre[:], in0=cp[:],
                                            in1=dinv_t[:, sl], op=ALU.mult)
                    nc.scalar.activation(out=tnew[:, sl], in_=pre[:], func=AF.Relu,
                                         accum_out=sums_t[:, b:b + 1])
                    sq_scr = trp.tile([P, P], dt.bfloat16)
                    nc.scalar.activation(out=sq_scr[:], in_=tnew[:, sl], func=AF.Square,
                                         accum_out=sqs_t[:, b:b + 1])

                # ---- BN stats + fold coefficients ----
                st = small.tile([P, 2], dt.float32)
                nc.vector.tensor_reduce(out=st[:, 0:1], in_=sums_t[:],
                                        axis=mybir.AxisListType.X, op=ALU.add)
                nc.vector.tensor_reduce(out=st[:, 1:2], in_=sqs_t[:],
                                        axis=mybir.AxisListType.X, op=ALU.add)
                st_in = dram.tile([P, 2], dt.float32)
                st_out = dram.tile([P, 2], dt.float32)
                nc.sync.dma_start(out=st_in[:], in_=st[:])
                if not SKIP_CC:
                    nc.gpsimd.collective_compute(
                        "AllReduce", ALU.add, replica_groups=rg,
                        ins=[st_in[:].opt()], outs=[st_out[:].opt()],
                    )
                else:
                    nc.sync.dma_start(out=st_out[:, :], in_=st_in[:, :])
                stg = small.tile([P, 2], dt.float32)
                nc.sync.dma_start(out=stg[:], in_=st_out[:])
                mu = small.tile([P, 1], dt.float32)
                nc.vector.tensor_scalar(out=mu[:], in0=stg[:, 0:1], scalar1=1.0 / N,
                                        scalar2=None, op0=ALU.mult)
                ex2 = small.tile([P, 1], dt.float32)
                nc.vector.tensor_scalar(out=ex2[:], in0=stg[:, 1:2], scalar1=1.0 / N,
                                        scalar2=None, op0=ALU.mult)
                var = small.tile([P, 1], dt.float32)
                nc.vector.tensor_tensor(out=var[:], in0=mu[:], in1=mu[:], op=ALU.mult)
                nc.vector.tensor_tensor(out=var[:], in0=ex2[:], in1=var[:],
                                        op=ALU.subtract)
                sd = small.tile([P, 1], dt.float32)
                nc.scalar.activation(out=sd[:], in_=var[:], func=AF.Sqrt, bias=eps_t[:])
                rs = small.tile([P, 1], dt.float32)
                nc.vector.reciprocal(out=rs[:], in_=sd[:])
                a_t = small.tile([P, 1], dt.float32)
                nc.vector.tensor_tensor(out=a_t[:], in0=rs[:], in1=g_t[k][:],
                                        op=ALU.mult)
                c_t = small.tile([P, 1], dt.float32)
                nc.vector.tensor_tensor(out=c_t[:], in0=mu[:], in1=a_t[:], op=ALU.mult)
                nc.vector.tensor_tensor(out=c_t[:], in0=bt_t[k][:], in1=c_t[:],
                                        op=ALU.subtract)
                bn_fold = (a_t, c_t)
                if DBG and k == 0:
                    nt = min(6 * P, NSH)
                    dbg_t1_sb = trp.tile([P, 6 * P], dt.bfloat16, name='dbg_t1_sb')
                    nc.vector.tensor_copy(out=dbg_t1_sb[:, :nt], in_=tnew[:, :nt])
                    nc.sync.dma_start(out=dbg_t1[:, :], in_=dbg_t1_sb[:])
                    dbg_st_sb = trp.tile([P, 6], dt.float32, name='dbg_st_sb')
                    for ii, tt in enumerate((mu, ex2, var, sd, a_t, c_t)):
                        nc.vector.tensor_copy(out=dbg_st_sb[:, ii:ii+1], in_=tt[:])
                    nc.sync.dma_start(out=dbg_stats[:, :], in_=dbg_st_sb[:])

            # ---- final linear: out^T = Wl'.T @ t3 + bl' ----
            a_t, c_t = bn_fold
            Wlp = small.tile([D, D], dt.bfloat16)
            nc.scalar.activation(out=Wlp[:], in_=Wl_t[:], func=AF.Identity, scale=a_t[:])
            pb = ps_tr.tile([1, D], dt.float32, space="PSUM",
                            padded_shape=[1, 512])
            nc.tensor.matmul(out=pb[:], lhsT=c_t[:], rhs=Wlf_t[:], start=True, stop=True)
            blp_row = small.tile([1, D], dt.float32)
            nc.vector.tensor_tensor(out=blp_row[:], in0=pb[:], in1=blrow_t[:], op=ALU.add)
            blp_bounce = dram.tile([1, D], dt.float32, name='blp_bounce')
            nc.sync.dma_start(out=blp_bounce[:], in_=blp_row[:])
            blp = small.tile([D, 1], dt.float32)
            nc.sync.dma_start(out=blp[:], in_=blp_bounce[0, :, None])
            for (o, w) in col_chunks:
                pp = ps_node.tile([P, w], dt.float32, space="PSUM",
                                     padded_shape=[P, 512])
                nc.tensor.matmul(out=pp[:], lhsT=Wlp[:], rhs=t_T[0][:, o:o + w],
                                 start=True, stop=True)
                ot = trp.tile([P, w], dt.float32)
                nc.scalar.activation(out=ot[:], in_=pp[:], func=AF.Identity, bias=blp[:])
                nc.sync.dma_start(out=outT[:, o:o + w], in_=ot[:])

    nc.finalize()
    return nc


# ---------------- public entry point ----------------

_CACHE = {}
LAST_EXEC_NS = None


def _make_in_maps(inputs, per_core, dinv):
    x = np.asarray(inputs["x"], np.float32)

    dinv_pad = np.zeros(NP, np.float32)
    dinv_pad[:N] = dinv
    xT_full = np.zeros((P, NP), np.float32)
    xT_full[:, :N] = x.T

    Ws = {k: np.asarray(inputs[k], np.float32) for k in
          ("W1", "W2", "W3", "Wl", "We1", "We2", "We3")}
    bt_tot = {k: np.asarray(inputs[f"b{k}"], np.float32) +
                 np.asarray(inputs[f"be{k}"], np.float32) for k in (1, 2, 3)}

    in_maps = []
    for c in range(NCORES):
        sl = slice(c * NSH, (c + 1) * NSH)
        im = dict(per_core[c])
        im["xT"] = _bf16(xT_full[:, sl])
        im["dinvt"] = _bf16(np.tile(dinv_pad[sl][None, :], (P, 1)))
        im["iota_u8"] = np.tile(np.arange(P, dtype=np.uint8)[None, :], (P, 1))
        im["ident"] = _bf16(np.eye(P))
        for i, k in enumerate((1, 2, 3)):
            im[f"W{k}"] = _bf16(Ws[f"W{k}"])
            Webd = np.zeros((4 * DE, 4 * D), np.float32)
            for j in range(4):
                Webd[16 * j:16 * j + 16, D * j:D * j + D] = Ws[f"We{k}"]
            im[f"We{k}"] = _bf16(Webd)
            im[f"g{k}"] = np.asarray(inputs[f"g{k}"], np.float32).reshape(D, 1)
            im[f"bt{k}"] = np.asarray(inputs[f"bt{k}"], np.float32).reshape(D, 1)
        im["Wf2"] = Ws["W2"]
        im["Wf3"] = Ws["W3"]
        im["Wlin"] = _bf16(Ws["Wl"])
        im["Wlinf"] = Ws["Wl"]
        im["bcol1"] = bt_tot[1].reshape(D, 1)
        im["brow2"] = bt_tot[2].reshape(1, D)
        im["brow3"] = bt_tot[3].reshape(1, D)
        im["blrow"] = np.asarray(inputs["bl"], np.float32).reshape(1, D)
        in_maps.append(im)
    return in_maps


def kernel(**inputs):
    edge_attr = np.asarray(inputs["edge_attr"], np.float32)
    edge_index = np.asarray(inputs["edge_index"])

    per_core, sched, dinv = _preprocess(edge_index, edge_attr)
    in_maps = _make_in_maps(inputs, per_core, dinv)

    key = ("k", sched["tot_chunks"], sched["tot_s"],
           tuple(sched["m_lo"]), tuple(sched["m_hi"]))
    if key not in _CACHE:
        _CACHE[key] = _build(sched)
    nc = _CACHE[key]

    import os
    trace = os.environ.get("KPROF") == "1"
    r = run_bass_kernel_spmd(nc, in_maps, core_ids=list(range(NCORES)), trace=trace)
    if trace:
        print(f"HW exec time: {r.exec_time_ns} ns", flush=True)
        global LAST_EXEC_NS
        LAST_EXEC_NS = r.exec_time_ns
        try:
            insts = r.instructions_and_trace[0] if r.instructions_and_trace else []
            import pickle
            rows = [
                dict(ts=i.timestamp, dur=i.duration, engine=str(i.engine),
                     name=i.name, label=i.label, wait=i.evt_wait_time,
                     bir=str(i.bir_str)[:200], src=f"{i.source_file}:{i.source_line}")
                for i in insts
            ]
            with open("/tmp/kprof_insts.pkl", "wb") as f:
                pickle.dump(rows, f)
            print(f"KPROF: dumped {len(rows)} insts; trace="
                  f"{r.instructions_and_trace[1] if r.instructions_and_trace else None}",
                  flush=True)
        except Exception as e:
            print(f"KPROF dump failed: {e}", flush=True)
    res = r.results
    outT = np.concatenate([res[c]["outT"] for c in range(NCORES)], axis=1)  # [128, NP]
    return np.ascontiguousarray(outT.T[:N]).astype(np.float32)



# revision 25
# speedup vs baseline: 2.6696x; 1.1206x over previous
"""GCN (3x GCNConv + BN + final linear) on 8 TRN2 NeuronCores.

Strategy (see test.py for the harness):
- Pad N=50000 -> NP=50176 = 392 blocks of 128 nodes. Core c owns 49
  blocks (6272 nodes) and all edges whose destination (col) lies in them.
- The GCN norm dinv[row]*dinv[col] is factorized: dinv[row] is folded into
  edge_attr (host) and into the gather table rows (device); dinv[col] is
  applied to the aggregated block output (device).
- BatchNorm+bias are affine per-feature and are folded into the next
  layer's weights on device, so the edge phase is just
  msg = relu(table[row] + ea' @ We); out[col] += msg,
  computed as dense matmuls: per 128-edge chunk an indicator one-hot
  matrix S (built on DVE from uint8 compares) scatters messages into a
  PSUM accumulator per destination block; gathers use the hardware
  dma_gather (int16 indices, table split in two 25088-row halves).
- Per layer: node linear (feature-major) -> AllGather bf16 table ->
  edge phase -> BN stats AllReduce -> fold affine into next weights.
"""

import sys

sys.path.insert(0, "/opt/trn_rl_repo")

import numpy as np
import ml_dtypes

import concourse.bass as bass
import concourse.tile as tile
from concourse import bacc, mybir
from concourse.bass_utils import run_bass_kernel_spmd

# ---------------- constants ----------------
NCORES = 8
D = 128
DE = 16
EPS = 1e-5
P = 128


def configure(n):
    """Set the node count; everything else derives from it."""
    global N, BLOCKS, NP, BPC, NSH, PA, PB, TABA, TABB
    N = n
    BLOCKS = ((N + P - 1) // P + NCORES - 1) // NCORES * NCORES
    NP = BLOCKS * P
    BPC = BLOCKS // NCORES
    NSH = BPC * P
    # Each core's shard is split into piece A (cols [0, PA)) and piece B
    # (cols [PA, NSH)); each piece is AllGathered separately so the second
    # collective overlaps the first piece's gathers. Both gather tables
    # stay under 2^15 rows (int16 gather indices).
    PA = (NSH // 2) // P // 2 * 2 * P          # 3072 for NSH=6272
    PB = NSH - PA
    TABA = NCORES * PA
    TABB = NCORES * PB


configure(50000)
E = 1_600_000

dt = mybir.dt
AF = mybir.ActivationFunctionType
ALU = mybir.AluOpType

S_DTYPE = dt.bfloat16        # indicator matrix dtype


def _bf16(a):
    return np.asarray(a, dtype=np.float32).astype(ml_dtypes.bfloat16)


# ---------------- host-side edge preprocessing ----------------

def _preprocess(edge_index, edge_attr):
    """Sort/pad edges per (core, dest-block); build packed device arrays.

    Returns dict of per-core numpy arrays + the uniform chunk schedule.
    """
    row = np.asarray(edge_index[0], dtype=np.int64)
    col = np.asarray(edge_index[1], dtype=np.int64)
    deg = np.bincount(row, minlength=N).astype(np.float32) + 1.0
    dinv = deg ** -0.5                                  # [N]
    ea_s = np.asarray(edge_attr, np.float32) * dinv[row][:, None]   # [E,16]

    blk = col // P                                      # dest block of each edge
    order = np.argsort(blk, kind="stable")
    row_s, col_s, blk_s = row[order], col[order], blk[order]
    ea_sorted = ea_s[order]
    # boundaries per block
    starts = np.searchsorted(blk_s, np.arange(BLOCKS))
    ends = np.searchsorted(blk_s, np.arange(BLOCKS), side="right")

    # per (core, local block): split rows by source piece (A: local col
    # < PA within the owner core's shard, B: the rest), sizes
    src_core = row_s // NSH
    src_local = row_s % NSH
    in_a = src_local < PA
    # gather-table row of each edge's source in its piece's table
    tab_row = np.where(in_a, src_core * PA + src_local,
                       src_core * PB + (src_local - PA))
    lists = [[None] * BPC for _ in range(NCORES)]
    n_lo = np.zeros((NCORES, BPC), np.int64)
    n_hi = np.zeros((NCORES, BPC), np.int64)
    for g in range(BLOCKS):
        c, b = divmod(g, BPC)
        s, e = starts[g], ends[g]
        lo_idx = np.nonzero(in_a[s:e])[0]
        hi_idx = np.nonzero(~in_a[s:e])[0]
        lists[c][b] = (s, lo_idx, hi_idx)
        n_lo[c, b] = len(lo_idx)
        n_hi[c, b] = len(hi_idx)

    # uniform chunk counts across cores
    m_lo = np.maximum(1, (n_lo.max(axis=0) + P - 1) // P).astype(int)   # [BPC]
    m_hi = np.maximum(1, (n_hi.max(axis=0) + P - 1) // P).astype(int)
    m_tot = m_lo + m_hi
    tot_chunks = int(m_tot.sum())
    chunk_off = np.zeros(BPC, int)
    chunk_off[1:] = np.cumsum(m_tot)[:-1]
    # 4-chunk groups for the batched (block-diag) edge-feature matmul
    n_grp = (m_tot + 3) // 4
    tot_grp = int(n_grp.sum())
    grp_off = np.zeros(BPC, int)
    grp_off[1:] = np.cumsum(n_grp)[:-1]
    # idx columns (16-wrapped) offsets, in units of int16 columns
    s_lo = m_lo * 8
    s_hi = m_hi * 8
    s_tot = s_lo + s_hi
    tot_s = int(s_tot.sum())
    s_off = np.zeros(BPC, int)
    s_off[1:] = np.cumsum(s_tot)[:-1]

    per_core = []
    for c in range(NCORES):
        eaT = np.zeros((DE, tot_chunks, P), np.float32)
        colrel = np.full((P, tot_chunks), 255, np.uint8)
        idx16 = np.zeros((16, tot_s), np.int16)
        for b in range(BPC):
            s, lo_idx, hi_idx = lists[c][b]
            co = chunk_off[b]
            for half, sub, m_half, half_chunk_base in (
                (0, lo_idx, m_lo[b], 0),
                (1, hi_idx, m_hi[b], m_lo[b]),
            ):
                g_sz = int(m_half) * P
                rows_h = np.zeros(g_sz, np.int64)        # pad idx -> 0
                rows_h[: len(sub)] = tab_row[s + sub]
                cols_h = np.full(g_sz, 255, np.int64)    # pad col -> 255
                cols_h[: len(sub)] = col_s[s + sub] - (c * BPC + b) * P
                ea_h = np.zeros((g_sz, DE), np.float32)
                ea_h[: len(sub)] = ea_sorted[s + sub]
                ii = np.arange(g_sz)
                pp, jj = ii % P, ii // P
                eaT[:, co + half_chunk_base + jj, pp] = ea_h.T
                colrel[pp, co + half_chunk_base + jj] = cols_h
                # 16-wrapped idx at column offset
                so = s_off[b] + (0 if half == 0 else s_lo[b])
                idx16[ii % 16, so + ii // 16] = rows_h
        # repack eaT [16, chunk, 128] -> [64, group, 128]: group g holds
        # chunks 4g..4g+3 of its block stacked along the partition axis, so
        # one matmul against a block-diagonal We computes e for 4 chunks.
        eaT_pack = np.zeros((4 * DE, tot_grp, P), np.float32)
        for b in range(BPC):
            m = int(m_tot[b])
            for j in range(m):
                eaT_pack[16 * (j % 4):16 * (j % 4) + 16, grp_off[b] + j // 4, :] = \
                    eaT[:, chunk_off[b] + j, :]
        per_core.append(
            dict(
                eaT=_bf16(eaT_pack),
                colrel=colrel,
                idx16=np.tile(idx16, (8, 1)),            # replicate to 128 partitions
            )
        )

    sched = dict(
        m_lo=[int(v) for v in m_lo], m_hi=[int(v) for v in m_hi],
        chunk_off=[int(v) for v in chunk_off], s_off=[int(v) for v in s_off],
        s_lo=[int(v) for v in s_lo],
        n_grp=[int(v) for v in n_grp], grp_off=[int(v) for v in grp_off],
        tot_chunks=tot_chunks, tot_s=tot_s, tot_grp=tot_grp,
    )
    return per_core, sched, dinv


# ---------------- device program ----------------

NQ = 4  # SWDGE queues: dma_gather queue q runs on Q7 core pair (2q, 2q+1),
        # so round-robin over 4 queues runs 4 gathers concurrently.


def _build(sched):
    nc = bacc.Bacc(None, target_bir_lowering=False, debug=False,
                   num_swdge_queues=NQ)
    TC, TS = sched["tot_chunks"], sched["tot_s"]
    TG = sched["tot_grp"]

    # ---- external inputs (per-core shapes) ----
    decl = nc.declare_dram_parameter
    xT = decl("xT", [P, NSH], dt.bfloat16, isOutput=False)
    eaT_d = decl("eaT", [4 * DE, TG, P], dt.bfloat16, isOutput=False)
    colrel_d = decl("colrel", [P, TC], dt.uint8, isOutput=False)
    idx_d = decl("idx16", [P, TS], dt.int16, isOutput=False)
    dinv_d = decl("dinvt", [P, NSH], dt.bfloat16, isOutput=False)
    iota_d = decl("iota_u8", [P, P], dt.uint8, isOutput=False)
    ident_d = decl("ident", [P, P], dt.bfloat16, isOutput=False)
    W_d = [decl(f"W{k}", [D, D], dt.bfloat16, isOutput=False) for k in (1, 2, 3)]
    Wf_d = [decl(f"Wf{k}", [D, D], dt.float32, isOutput=False) for k in (2, 3)]
    Wl_d = decl("Wlin", [D, D], dt.bfloat16, isOutput=False)
    Wlf_d = decl("Wlinf", [D, D], dt.float32, isOutput=False)
    We_d = [decl(f"We{k}", [4 * DE, 4 * D], dt.bfloat16, isOutput=False)
            for k in (1, 2, 3)]  # block-diag: 4 copies of We on the diagonal
    # b_tot[k] = b_k + be_k as column [128,1]; rows [1,128] for fold matmuls
    bcol1_d = decl("bcol1", [D, 1], dt.float32, isOutput=False)
    brow_d = [decl(f"brow{k}", [1, D], dt.float32, isOutput=False) for k in (2, 3)]
    blrow_d = decl("blrow", [1, D], dt.float32, isOutput=False)
    g_d = [decl(f"g{k}", [D, 1], dt.float32, isOutput=False) for k in (1, 2, 3)]
    bt_d = [decl(f"bt{k}", [D, 1], dt.float32, isOutput=False) for k in (1, 2, 3)]
    outT = decl("outT", [P, NSH], dt.float32, isOutput=True)
    import os
    DBG = os.environ.get("KDBG") == "1"
    SKIP_GATHER = os.environ.get("KSKIP_GATHER") == "1"
    SKIP_CC = os.environ.get("KSKIP_CC") == "1"
    SKIP_EDGE = os.environ.get("KSKIP_EDGE") == "1"
    SKIP_EA = os.environ.get("KSKIP_EA") == "1"
    if DBG:
        dbg_table = decl("dbg_table", [NP, D], dt.bfloat16, isOutput=True)
        dbg_hr = decl("dbg_hr", [P, 4 * D], dt.bfloat16, isOutput=True)
        dbg_msg = decl("dbg_msg", [P, 4 * D], dt.bfloat16, isOutput=True)
        dbg_t1 = decl("dbg_t1", [P, 6 * P], dt.bfloat16, isOutput=True)
        dbg_stats = decl("dbg_stats", [P, 6], dt.float32, isOutput=True)
        dbg_table2 = decl("dbg_table2", [NP, D], dt.bfloat16, isOutput=True)

    rg = [list(range(NCORES))]

    with tile.TileContext(nc) as tc:
        import contextlib
        with contextlib.ExitStack() as ctx:
            ek = ctx.enter_context
            const = ek(tc.tile_pool(name="const", bufs=1))
            nodeb = ek(tc.tile_pool(name="nodeb", bufs=2))
            edge_ea = ek(tc.tile_pool(name="edge_ea", bufs=3))
            edge_idx = ek(tc.tile_pool(name="edge_idx", bufs=4))
            edge_hr = ek(tc.tile_pool(name="edge_hr", bufs=3))
            edge_msg = ek(tc.tile_pool(name="edge_msg", bufs=2))
            edge_S = ek(tc.tile_pool(name="edge_S", bufs=3))
            small = ek(tc.tile_pool(name="small", bufs=4))
            trp = ek(tc.tile_pool(name="trp", bufs=3))
            ps_mp = ek(tc.tile_pool(name="ps_mp", bufs=3, space="PSUM"))
            ps_conv = ek(tc.tile_pool(name="ps_conv", bufs=2, space="PSUM"))
            ps_node = ek(tc.tile_pool(name="ps_node", bufs=1, space="PSUM"))
            ps_tr = ek(tc.tile_pool(name="ps_tr", bufs=1, space="PSUM"))
            dram = ek(tc.tile_pool(name="dram", bufs=2, space="DRAM"))

            # ---- load constants ----
            def ld(pool, shape, dty, src, name):
                t = pool.tile(shape, dty, name=name)
                nc.sync.dma_start(out=t[:], in_=src[...])
                return t

            xT_t = ld(const, [P, NSH], dt.bfloat16, xT, 'xT_t')
            dinv_t = ld(const, [P, NSH], dt.bfloat16, dinv_d, 'dinv_d_t')
            iota_t = ld(const, [P, P], dt.uint8, iota_d, 'iota_d_t')
            ident_t = ld(const, [P, P], dt.bfloat16, ident_d, 'ident_d_t')
            colrel_t = ld(const, [P, TC], dt.uint8, colrel_d, 'colrel_d_t')
            W_t = [ld(const, [D, D], dt.bfloat16, W_d[i], f'W_t{i}') for i in range(3)]
            Wf_t = [ld(const, [D, D], dt.float32, Wf_d[i], f'Wf_t{i}') for i in range(2)]
            Wl_t = ld(const, [D, D], dt.bfloat16, Wl_d, 'Wl_d_t')
            Wlf_t = ld(const, [D, D], dt.float32, Wlf_d, 'Wlf_d_t')
            We_t = [ld(const, [4 * DE, 4 * D], dt.bfloat16, We_d[i], f'We_t{i}')
                    for i in range(3)]
            bcol1_t = ld(const, [D, 1], dt.float32, bcol1_d, 'bcol1_d_t')
            brow_t = [ld(const, [1, D], dt.float32, brow_d[i], f'brow_t{i}') for i in range(2)]
            blrow_t = ld(const, [1, D], dt.float32, blrow_d, 'blrow_d_t')
            g_t = [ld(const, [D, 1], dt.float32, g_d[i], f'g_t{i}') for i in range(3)]
            bt_t = [ld(const, [D, 1], dt.float32, bt_d[i], f'bt_t{i}') for i in range(3)]

            t_T = [const.tile([P, NSH], dt.bfloat16, name=f't_T{i}') for i in range(2)]
            eps_t = const.tile([P, 1], dt.float32, name='eps_t')
            nc.vector.memset(eps_t[:], EPS)

            m_lo, m_hi = sched["m_lo"], sched["m_hi"]
            chunk_off, s_off, s_lo = sched["chunk_off"], sched["s_off"], sched["s_lo"]
            n_grp, grp_off = sched["n_grp"], sched["grp_off"]
            qctr = [0]  # round-robin SWDGE queue for dma_gather

            col_chunks = [(o, min(512, NSH - o)) for o in range(0, NSH, 512)]
            col_chunks_a = [(o, w) for (o, w) in col_chunks if o < PA]
            col_chunks_b = [(o, w) for (o, w) in col_chunks if o >= PA]

            def node_linear(rhs_t, Wp_t, bp_t, hout_t, chunks):
                """hout = dinv * (W'.T @ rhs + b')  (feature-major)."""
                for (o, w) in chunks:
                    pp = ps_node.tile([P, w], dt.float32, space="PSUM",
                                         padded_shape=[P, 512])
                    nc.tensor.matmul(out=pp[:], lhsT=Wp_t[:], rhs=rhs_t[:, o:o + w],
                                     start=True, stop=True)
                    tmp = trp.tile([P, w], dt.bfloat16)
                    nc.scalar.activation(out=tmp[:], in_=pp[:], func=AF.Identity,
                                         bias=bp_t[:])
                    nc.vector.tensor_tensor(out=hout_t[:, o:o + w], in0=tmp[:],
                                            in1=dinv_t[:, o:o + w], op=ALU.mult)

            for k in range(3):  # layers 1..3
                # ---- fold previous BN (k>=1) into this layer's weights ----
                if k == 0:
                    Wp_t, bp_t = W_t[0], bcol1_t
                    rhs_t = xT_t
                else:
                    a_t, c_t = bn_fold  # from previous layer epilogue
                    Wp_t = small.tile([D, D], dt.bfloat16)
                    nc.scalar.activation(out=Wp_t[:], in_=W_t[k][:], func=AF.Identity,
                                         scale=a_t[:])
                    pb = ps_tr.tile([1, D], dt.float32, space="PSUM",
                                    padded_shape=[1, 512])
                    nc.tensor.matmul(out=pb[:], lhsT=c_t[:], rhs=Wf_t[k - 1][:],
                                     start=True, stop=True)
                    bprow = small.tile([1, D], dt.float32)
                    nc.vector.tensor_tensor(out=bprow[:], in0=pb[:], in1=brow_t[k - 1][:],
                                            op=ALU.add)
                    bp_bounce = dram.tile([1, D], dt.float32, name='bp_bounce')
                    nc.sync.dma_start(out=bp_bounce[:], in_=bprow[:])
                    bp_t = small.tile([D, 1], dt.float32)
                    nc.sync.dma_start(out=bp_t[:], in_=bp_bounce[0, :, None])
                    rhs_t = t_T[(k - 1) % 2]

                # ---- node linear + transpose + AllGather, piece by piece ----
                # Piece A's AllGather is issued as soon as its shard is
                # written, so it overlaps piece B's node compute; AG_B then
                # overlaps the early piece-A gathers of the edge phase.
                hlin_t = nodeb.tile([P, NSH], dt.bfloat16)
                shard_a = dram.tile([PA, D], dt.bfloat16)
                shard_b = dram.tile([PB, D], dt.bfloat16)
                table_a = dram.tile([TABA, D], dt.bfloat16)
                table_b = dram.tile([TABB, D], dt.bfloat16)
                BA = PA // P
                for (chunks, t0, t1, shard, table) in (
                    (col_chunks_a, 0, BA, shard_a, table_a),
                    (col_chunks_b, BA, BPC, shard_b, table_b),
                ):
                    node_linear(rhs_t, Wp_t, bp_t, hlin_t, chunks)
                    for t in range(t0, t1):
                        ptr = ps_tr.tile([P, P], dt.bfloat16, space="PSUM",
                                         padded_shape=[P, 1024])
                        nc.tensor.transpose(out=ptr[:], in_=hlin_t[:, t * P:(t + 1) * P],
                                            identity=ident_t[:])
                        sb = trp.tile([P, P], dt.bfloat16)
                        nc.scalar.activation(out=sb[:], in_=ptr[:], func=AF.Copy)
                        nc.sync.dma_start(out=shard[(t - t0) * P:(t - t0 + 1) * P, :],
                                          in_=sb[:])
                    if not SKIP_CC:
                        nc.gpsimd.collective_compute(
                            "AllGather", ALU.bypass, replica_groups=rg,
                            ins=[shard[:].opt()], outs=[table[:].opt()],
                        )
                    else:
                        nc.sync.dma_start(out=table[:shard.shape[0], :], in_=shard[:, :])

                # ---- edge phase over 49 dest blocks ----
                sums_t = small.tile([P, BPC], dt.float32)
                sqs_t = small.tile([P, BPC], dt.float32)
                tnew = t_T[k % 2]
                for b in range(BPC):
                    m = m_lo[b] + m_hi[b]
                    co = chunk_off[b]
                    ng, go = n_grp[b], grp_off[b]
                    ea_t = edge_ea.tile([4 * DE, ng, P], dt.bfloat16)
                    if not SKIP_EA:
                        nc.sync.dma_start(out=ea_t[:], in_=eaT_d[:, go:go + ng, :])
                    else:
                        nc.vector.memset(ea_t[:, :1, :], 0.0)
                    stot_b = s_lo[b] + m_hi[b] * 8
                    idx_t = edge_idx.tile([P, stot_b], dt.int16, name=f'idxb')
                    nc.sync.dma_start(out=idx_t[:], in_=idx_d[:, s_off[b]:s_off[b] + stot_b])
                    hr_t = edge_hr.tile([P, m, D], dt.bfloat16)
                    # Gather per (block, table-half), split into balanced
                    # pieces of <=8 chunks (1024 idxs = 65 descs/engine —
                    # the SWDGE ring rejects larger calls; 81 and 137
                    # descs/engine both failed on HW). queue q executes on
                    # Q7 core pair (2q, 2q+1); cycling queues runs 4
                    # gathers' desc-gen concurrently.
                    if not SKIP_GATHER:
                        for mh, tab, mbase, sbase in (
                            (m_lo[b], table_a[:, :], 0, 0),
                            (m_hi[b], table_b[:, :], m_lo[b], s_lo[b]),
                        ):
                            npc = (mh + 7) // 8
                            step = (mh + npc - 1) // npc
                            for pc in range(0, mh, step):
                                pw = min(step, mh - pc)
                                nc.gpsimd.dma_gather(
                                    out_ap=hr_t[:, mbase + pc:mbase + pc + pw, :],
                                    in_ap=tab,
                                    idxs_ap=idx_t[:, sbase + pc * 8:sbase + (pc + pw) * 8],
                                    num_idxs=pw * P, num_idxs_reg=pw * P, elem_size=D,
                                    queue_num=qctr[0],
                                )
                                qctr[0] = (qctr[0] + 1) % NQ
                    else:
                        nc.vector.memset(hr_t[:, :1, :], 0.25)
                    if DBG and k == 0 and b == 0:
                        dbg_hr_sb = trp.tile([P, 4 * D], dt.bfloat16, name='dbg_hr_sb')
                        nc.vector.tensor_copy(out=dbg_hr_sb[:], in_=hr_t[:, :4, :].rearrange("p m d -> p (m d)"))
                        nc.sync.dma_start(out=dbg_hr[:, :], in_=dbg_hr_sb[:])
                    # S indicator [P, m, P]
                    S_t = edge_S.tile([P, m, P], S_DTYPE)
                    iota_b = bass.AP(tensor=iota_t.tensor, offset=iota_t[:].offset,
                                     ap=[iota_t[:].ap[0], [0, m], iota_t[:].ap[1]])
                    cr = colrel_t[:, co:co + m]
                    cr_b = bass.AP(tensor=colrel_t.tensor, offset=cr.offset,
                                   ap=[cr.ap[0], cr.ap[1], [0, P]])
                    nc.vector.tensor_tensor(out=S_t[:], in0=iota_b, in1=cr_b,
                                            op=ALU.is_equal)
                    # messages: per 4-chunk group, identity-inject hr then one
                    # block-diag We matmul computes e for all 4 chunks.
                    msg_t = edge_msg.tile([P, m, D], dt.bfloat16)
                    if SKIP_EDGE:
                        nc.vector.memset(msg_t[:, :1, :], 0.1)
                    for g in range(ng if not SKIP_EDGE else 0):
                        j = 4 * g
                        jw = min(4, m - j)
                        mp = ps_mp.tile([P, 4, D], dt.float32, space="PSUM")
                        # start=True zeroes the whole 2KB bank, so the
                        # full-tile identity-add must come first.
                        nc.tensor.matmul(
                            out=mp[:, :jw, :].rearrange("p j d -> p (j d)"),
                            lhsT=ident_t[:],
                            rhs=hr_t[:, j:j + jw, :].rearrange("p j d -> p (j d)"),
                            start=True, stop=False, skip_group_check=True)
                        nc.tensor.matmul(
                            out=mp[:, :jw, :].rearrange("p j d -> p (j d)"),
                            lhsT=ea_t[:16 * jw, g, :],
                            rhs=We_t[k][:16 * jw, :jw * D],
                            start=False, stop=True, skip_group_check=True)
                        nc.scalar.activation(
                            out=msg_t[:, j:j + jw, :].rearrange("p j d -> p (j d)"),
                            in_=mp[:, :jw, :].rearrange("p j d -> p (j d)"), func=AF.Relu)
                    if DBG and k == 0 and b == 0:
                        dbg_msg_sb = trp.tile([P, 4 * D], dt.bfloat16, name='dbg_msg_sb')
                        nc.vector.tensor_copy(out=dbg_msg_sb[:], in_=msg_t[:, :4, :].rearrange("p m d -> p (m d)"))
                        nc.sync.dma_start(out=dbg_msg[:, :], in_=dbg_msg_sb[:])
                    # scatter into conv accumulator (feature-major out)
                    cp = ps_conv.tile([P, P], dt.float32, space="PSUM",
                                      padded_shape=[P, 512])
                    for j in range(m if not SKIP_EDGE else 1):
                        nc.tensor.matmul(out=cp[:], lhsT=msg_t[:, j, :],
                                         rhs=S_t[:, j, :],
                                         start=(j == 0), stop=(j == m - 1))
                    # epilogue: dinv scale, relu -> t, stats
                    sl = slice(b * P, (b + 1) * P)
                    pre = trp.tile([P, P], dt.float32)
                    nc.vector.tensor_tensor(out=pre[:], in0=cp[:],
                                            in1=dinv_t[:, sl], op=ALU.mult)
                    nc.scalar.activation(out=tnew[:, sl], in_=pre[:], func=AF.Relu,
                                         accum_out=sums_t[:, b:b + 1])
                    sq_scr = trp.tile([P, P], dt.bfloat16)
                    nc.scalar.activation(out=sq_scr[:], in_=tnew[:, sl], func=AF.Square,
                                         accum_out=sqs_t[:, b:b + 1])

                # ---- BN stats + fold coefficients ----
                st = small.tile([P, 2], dt.float32)
                nc.vector.tensor_reduce(out=st[:, 0:1], in_=sums_t[:],
                                        axis=mybir.AxisListType.X, op=ALU.add)
                nc.vector.tensor_reduce(out=st[:, 1:2], in_=sqs_t[:],
                                        axis=mybir.AxisListType.X, op=ALU.add)
                st_in = dram.tile([P, 2], dt.float32)
                st_out = dram.tile([P, 2], dt.float32)
                nc.sync.dma_start(out=st_in[:], in_=st[:])
                if not SKIP_CC:
                    nc.gpsimd.collective_compute(
                        "AllReduce", ALU.add, replica_groups=rg,
                        ins=[st_in[:].opt()], outs=[st_out[:].opt()],
                    )
                else:
                    nc.sync.dma_start(out=st_out[:, :], in_=st_in[:, :])
                stg = small.tile([P, 2], dt.float32)
                nc.sync.dma_start(out=stg[:], in_=st_out[:])
                mu = small.tile([P, 1], dt.float32)
                nc.vector.tensor_scalar(out=mu[:], in0=stg[:, 0:1], scalar1=1.0 / N,
                                        scalar2=None, op0=ALU.mult)
                ex2 = small.tile([P, 1], dt.float32)
                nc.vector.tensor_scalar(out=ex2[:], in0=stg[:, 1:2], scalar1=1.0 / N,
                                        scalar2=None, op0=ALU.mult)
                var = small.tile([P, 1], dt.float32)
                nc.vector.tensor_tensor(out=var[:], in0=mu[:], in1=mu[:], op=ALU.mult)
                nc.vector.tensor_tensor(out=var[:], in0=ex2[:], in1=var[:],
                                        op=ALU.subtract)
                sd = small.tile([P, 1], dt.float32)
                nc.scalar.activation(out=sd[:], in_=var[:], func=AF.Sqrt, bias=eps_t[:])
                rs = small.tile([P, 1], dt.float32)
                nc.vector.reciprocal(out=rs[:], in_=sd[:])
                a_t = small.tile([P, 1], dt.float32)
                nc.vector.tensor_tensor(out=a_t[:], in0=rs[:], in1=g_t[k][:],
                                        op=ALU.mult)
                c_t = small.tile([P, 1], dt.float32)
                nc.vector.tensor_tensor(out=c_t[:], in0=mu[:], in1=a_t[:], op=ALU.mult)
                nc.vector.tensor_tensor(out=c_t[:], in0=bt_t[k][:], in1=c_t[:],
                                        op=ALU.subtract)
                bn_fold = (a_t, c_t)
                if DBG and k == 0:
                    nt = min(6 * P, NSH)
                    dbg_t1_sb = trp.tile([P, 6 * P], dt.bfloat16, name='dbg_t1_sb')
                    nc.vector.tensor_copy(out=dbg_t1_sb[:, :nt], in_=tnew[:, :nt])
                    nc.sync.dma_start(out=dbg_t1[:, :], in_=dbg_t1_sb[:])
                    dbg_st_sb = trp.tile([P, 6], dt.float32, name='dbg_st_sb')
                    for ii, tt in enumerate((mu, ex2, var, sd, a_t, c_t)):
                        nc.vector.tensor_copy(out=dbg_st_sb[:, ii:ii+1], in_=tt[:])
                    nc.sync.dma_start(out=dbg_stats[:, :], in_=dbg_st_sb[:])

            # ---- final linear: out^T = Wl'.T @ t3 + bl' ----
            a_t, c_t = bn_fold
            Wlp = small.tile([D, D], dt.bfloat16)
            nc.scalar.activation(out=Wlp[:], in_=Wl_t[:], func=AF.Identity, scale=a_t[:])
            pb = ps_tr.tile([1, D], dt.float32, space="PSUM",
                            padded_shape=[1, 512])
            nc.tensor.matmul(out=pb[:], lhsT=c_t[:], rhs=Wlf_t[:], start=True, stop=True)
            blp_row = small.tile([1, D], dt.float32)
            nc.vector.tensor_tensor(out=blp_row[:], in0=pb[:], in1=blrow_t[:], op=ALU.add)
            blp_bounce = dram.tile([1, D], dt.float32, name='blp_bounce')
            nc.sync.dma_start(out=blp_bounce[:], in_=blp_row[:])
            blp = small.tile([D, 1], dt.float32)
            nc.sync.dma_start(out=blp[:], in_=blp_bounce[0, :, None])
            for (o, w) in col_chunks:
                pp = ps_node.tile([P, w], dt.float32, space="PSUM",
                                     padded_shape=[P, 512])
                nc.tensor.matmul(out=pp[:], lhsT=Wlp[:], rhs=t_T[0][:, o:o + w],
                                 start=True, stop=True)
                ot = trp.tile([P, w], dt.float32)
                nc.scalar.activation(out=ot[:], in_=pp[:], func=AF.Identity, bias=blp[:])
                nc.sync.dma_start(out=outT[:, o:o + w], in_=ot[:])

    nc.finalize()
    return nc


# ---------------- public entry point ----------------

_CACHE = {}
LAST_EXEC_NS = None


def _make_in_maps(inputs, per_core, dinv):
    x = np.asarray(inputs["x"], np.float32)

    dinv_pad = np.zeros(NP, np.float32)
    dinv_pad[:N] = dinv
    xT_full = np.zeros((P, NP), np.float32)
    xT_full[:, :N] = x.T

    Ws = {k: np.asarray(inputs[k], np.float32) for k in
          ("W1", "W2", "W3", "Wl", "We1", "We2", "We3")}
    bt_tot = {k: np.asarray(inputs[f"b{k}"], np.float32) +
                 np.asarray(inputs[f"be{k}"], np.float32) for k in (1, 2, 3)}

    in_maps = []
    for c in range(NCORES):
        sl = slice(c * NSH, (c + 1) * NSH)
        im = dict(per_core[c])
        im["xT"] = _bf16(xT_full[:, sl])
        im["dinvt"] = _bf16(np.tile(dinv_pad[sl][None, :], (P, 1)))
        im["iota_u8"] = np.tile(np.arange(P, dtype=np.uint8)[None, :], (P, 1))
        im["ident"] = _bf16(np.eye(P))
        for i, k in enumerate((1, 2, 3)):
            im[f"W{k}"] = _bf16(Ws[f"W{k}"])
            Webd = np.zeros((4 * DE, 4 * D), np.float32)
            for j in range(4):
                Webd[16 * j:16 * j + 16, D * j:D * j + D] = Ws[f"We{k}"]
            im[f"We{k}"] = _bf16(Webd)
            im[f"g{k}"] = np.asarray(inputs[f"g{k}"], np.float32).reshape(D, 1)
            im[f"bt{k}"] = np.asarray(inputs[f"bt{k}"], np.float32).reshape(D, 1)
        im["Wf2"] = Ws["W2"]
        im["Wf3"] = Ws["W3"]
        im["Wlin"] = _bf16(Ws["Wl"])
        im["Wlinf"] = Ws["Wl"]
        im["bcol1"] = bt_tot[1].reshape(D, 1)
        im["brow2"] = bt_tot[2].reshape(1, D)
        im["brow3"] = bt_tot[3].reshape(1, D)
        im["blrow"] = np.asarray(inputs["bl"], np.float32).reshape(1, D)
        in_maps.append(im)
    return in_maps


def kernel(**inputs):
    edge_attr = np.asarray(inputs["edge_attr"], np.float32)
    edge_index = np.asarray(inputs["edge_index"])

    per_core, sched, dinv = _preprocess(edge_index, edge_attr)
    in_maps = _make_in_maps(inputs, per_core, dinv)

    key = ("k", sched["tot_chunks"], sched["tot_s"],
           tuple(sched["m_lo"]), tuple(sched["m_hi"]))
    if key not in _CACHE:
        _CACHE[key] = _build(sched)
    nc = _CACHE[key]

    import os
    trace = os.environ.get("KPROF") == "1"
    r = run_bass_kernel_spmd(nc, in_maps, core_ids=list(range(NCORES)), trace=trace)
    if trace:
        print(f"HW exec time: {r.exec_time_ns} ns", flush=True)
        global LAST_EXEC_NS
        LAST_EXEC_NS = r.exec_time_ns
        try:
            insts = r.instructions_and_trace[0] if r.instructions_and_trace else []
            import pickle
            rows = [
                dict(ts=i.timestamp, dur=i.duration, engine=str(i.engine),
                     name=i.name, label=i.label, wait=i.evt_wait_time,
                     bir=str(i.bir_str)[:200], src=f"{i.source_file}:{i.source_line}")
                for i in insts
            ]
            with open("/tmp/kprof_insts.pkl", "wb") as f:
                pickle.dump(rows, f)
            print(f"KPROF: dumped {len(rows)} insts; trace="
                  f"{r.instructions_and_trace[1] if r.instructions_and_trace else None}",
                  flush=True)
        except Exception as e:
            print(f"KPROF dump failed: {e}", flush=True)
    res = r.results
    outT = np.concatenate([res[c]["outT"] for c in range(NCORES)], axis=1)  # [128, NP]
    return np.ascontiguousarray(outT.T[:N]).astype(np.float32)



# revision 30
# speedup vs baseline: 2.6914x; 1.0082x over previous
"""GCN (3x GCNConv + BN + final linear) on 8 TRN2 NeuronCores.

Strategy (see test.py for the harness):
- Pad N=50000 -> NP=50176 = 392 blocks of 128 nodes. Core c owns 49
  blocks (6272 nodes) and all edges whose destination (col) lies in them.
- The GCN norm dinv[row]*dinv[col] is factorized: dinv[row] is folded into
  edge_attr (host) and into the gather table rows (device); dinv[col] is
  applied to the aggregated block output (device).
- BatchNorm+bias are affine per-feature and are folded into the next
  layer's weights on device, so the edge phase is just
  msg = relu(table[row] + ea' @ We); out[col] += msg,
  computed as dense matmuls: per 128-edge chunk an indicator one-hot
  matrix S (built on DVE from uint8 compares) scatters messages into a
  PSUM accumulator per destination block; gathers use the hardware
  dma_gather (int16 indices, table split in two 25088-row halves).
- Per layer: node linear (feature-major) -> AllGather bf16 table ->
  edge phase -> BN stats AllReduce -> fold affine into next weights.
"""

import sys

sys.path.insert(0, "/opt/trn_rl_repo")

import numpy as np
import ml_dtypes

import concourse.bass as bass
import concourse.tile as tile
from concourse import bacc, mybir
from concourse.bass_utils import run_bass_kernel_spmd

# ---------------- constants ----------------
NCORES = 8
D = 128
DE = 16
EPS = 1e-5
P = 128


def configure(n):
    """Set the node count; everything else derives from it."""
    global N, BLOCKS, NP, BPC, NSH, PA, PB, TABA, TABB
    N = n
    BLOCKS = ((N + P - 1) // P + NCORES - 1) // NCORES * NCORES
    NP = BLOCKS * P
    BPC = BLOCKS // NCORES
    NSH = BPC * P
    # Each core's shard is split into piece A (cols [0, PA)) and piece B
    # (cols [PA, NSH)); each piece is AllGathered separately so the second
    # collective overlaps the first piece's gathers. Both gather tables
    # stay under 2^15 rows (int16 gather indices).
    PA = (NSH // 2) // P // 2 * 2 * P          # 3072 for NSH=6272
    PB = NSH - PA
    TABA = NCORES * PA
    TABB = NCORES * PB


configure(50000)
E = 1_600_000

dt = mybir.dt
AF = mybir.ActivationFunctionType
ALU = mybir.AluOpType

S_DTYPE = dt.bfloat16        # indicator matrix dtype


def _bf16(a):
    return np.asarray(a, dtype=np.float32).astype(ml_dtypes.bfloat16)


# ---------------- host-side edge preprocessing ----------------

def _preprocess(edge_index, edge_attr):
    """Sort/pad edges per (core, dest-block); build packed device arrays.

    Returns dict of per-core numpy arrays + the uniform chunk schedule.
    """
    row = np.asarray(edge_index[0], dtype=np.int64)
    col = np.asarray(edge_index[1], dtype=np.int64)
    deg = np.bincount(row, minlength=N).astype(np.float32) + 1.0
    dinv = deg ** -0.5                                  # [N]
    ea_s = np.asarray(edge_attr, np.float32) * dinv[row][:, None]   # [E,16]

    blk = col // P                                      # dest block of each edge
    order = np.argsort(blk, kind="stable")
    row_s, col_s, blk_s = row[order], col[order], blk[order]
    ea_sorted = ea_s[order]
    # boundaries per block
    starts = np.searchsorted(blk_s, np.arange(BLOCKS))
    ends = np.searchsorted(blk_s, np.arange(BLOCKS), side="right")

    # per (core, local block): split rows by source piece (A: local col
    # < PA within the owner core's shard, B: the rest), sizes
    src_core = row_s // NSH
    src_local = row_s % NSH
    in_a = src_local < PA
    # gather-table row of each edge's source in its piece's table
    tab_row = np.where(in_a, src_core * PA + src_local,
                       src_core * PB + (src_local - PA))
    lists = [[None] * BPC for _ in range(NCORES)]
    n_lo = np.zeros((NCORES, BPC), np.int64)
    n_hi = np.zeros((NCORES, BPC), np.int64)
    for g in range(BLOCKS):
        c, b = divmod(g, BPC)
        s, e = starts[g], ends[g]
        lo_idx = np.nonzero(in_a[s:e])[0]
        hi_idx = np.nonzero(~in_a[s:e])[0]
        lists[c][b] = (s, lo_idx, hi_idx)
        n_lo[c, b] = len(lo_idx)
        n_hi[c, b] = len(hi_idx)

    # uniform chunk counts across cores
    m_lo = np.maximum(1, (n_lo.max(axis=0) + P - 1) // P).astype(int)   # [BPC]
    m_hi = np.maximum(1, (n_hi.max(axis=0) + P - 1) // P).astype(int)
    m_tot = m_lo + m_hi
    tot_chunks = int(m_tot.sum())
    chunk_off = np.zeros(BPC, int)
    chunk_off[1:] = np.cumsum(m_tot)[:-1]
    # 4-chunk groups for the batched (block-diag) edge-feature matmul
    n_grp = (m_tot + 3) // 4
    tot_grp = int(n_grp.sum())
    grp_off = np.zeros(BPC, int)
    grp_off[1:] = np.cumsum(n_grp)[:-1]
    # idx columns (16-wrapped) offsets, in units of int16 columns
    s_lo = m_lo * 8
    s_hi = m_hi * 8
    s_tot = s_lo + s_hi
    tot_s = int(s_tot.sum())
    s_off = np.zeros(BPC, int)
    s_off[1:] = np.cumsum(s_tot)[:-1]

    per_core = []
    for c in range(NCORES):
        eaT = np.zeros((DE, tot_chunks, P), np.float32)
        colrel = np.full((P, tot_chunks), 255, np.uint8)
        idx16 = np.zeros((16, tot_s), np.int16)
        for b in range(BPC):
            s, lo_idx, hi_idx = lists[c][b]
            co = chunk_off[b]
            for half, sub, m_half, half_chunk_base in (
                (0, lo_idx, m_lo[b], 0),
                (1, hi_idx, m_hi[b], m_lo[b]),
            ):
                g_sz = int(m_half) * P
                rows_h = np.zeros(g_sz, np.int64)        # pad idx -> 0
                rows_h[: len(sub)] = tab_row[s + sub]
                cols_h = np.full(g_sz, 255, np.int64)    # pad col -> 255
                cols_h[: len(sub)] = col_s[s + sub] - (c * BPC + b) * P
                ea_h = np.zeros((g_sz, DE), np.float32)
                ea_h[: len(sub)] = ea_sorted[s + sub]
                ii = np.arange(g_sz)
                pp, jj = ii % P, ii // P
                eaT[:, co + half_chunk_base + jj, pp] = ea_h.T
                colrel[pp, co + half_chunk_base + jj] = cols_h
                # 16-wrapped idx at column offset
                so = s_off[b] + (0 if half == 0 else s_lo[b])
                idx16[ii % 16, so + ii // 16] = rows_h
        # repack eaT [16, chunk, 128] -> [64, group, 128]: group g holds
        # chunks 4g..4g+3 of its block stacked along the partition axis, so
        # one matmul against a block-diagonal We computes e for 4 chunks.
        eaT_pack = np.zeros((4 * DE, tot_grp, P), np.float32)
        for b in range(BPC):
            m = int(m_tot[b])
            for j in range(m):
                eaT_pack[16 * (j % 4):16 * (j % 4) + 16, grp_off[b] + j // 4, :] = \
                    eaT[:, chunk_off[b] + j, :]
        per_core.append(
            dict(
                eaT=_bf16(eaT_pack),
                colrel=colrel,
                idx16=np.tile(idx16, (8, 1)),            # replicate to 128 partitions
            )
        )

    sched = dict(
        m_lo=[int(v) for v in m_lo], m_hi=[int(v) for v in m_hi],
        chunk_off=[int(v) for v in chunk_off], s_off=[int(v) for v in s_off],
        s_lo=[int(v) for v in s_lo],
        n_grp=[int(v) for v in n_grp], grp_off=[int(v) for v in grp_off],
        tot_chunks=tot_chunks, tot_s=tot_s, tot_grp=tot_grp,
    )
    return per_core, sched, dinv


# ---------------- device program ----------------

NQ = 4  # SWDGE queues: dma_gather queue q runs on Q7 core pair (2q, 2q+1),
        # so round-robin over 4 queues runs 4 gathers concurrently.


def _build(sched):
    nc = bacc.Bacc(None, target_bir_lowering=False, debug=False,
                   num_swdge_queues=NQ)
    TC, TS = sched["tot_chunks"], sched["tot_s"]
    TG = sched["tot_grp"]

    # ---- external inputs (per-core shapes) ----
    decl = nc.declare_dram_parameter
    xT = decl("xT", [P, NSH], dt.bfloat16, isOutput=False)
    eaT_d = decl("eaT", [4 * DE, TG, P], dt.bfloat16, isOutput=False)
    colrel_d = decl("colrel", [P, TC], dt.uint8, isOutput=False)
    idx_d = decl("idx16", [P, TS], dt.int16, isOutput=False)
    dinv_d = decl("dinvt", [P, NSH], dt.bfloat16, isOutput=False)
    iota_d = decl("iota_u8", [P, P], dt.uint8, isOutput=False)
    ident_d = decl("ident", [P, P], dt.bfloat16, isOutput=False)
    W_d = [decl(f"W{k}", [D, D], dt.bfloat16, isOutput=False) for k in (1, 2, 3)]
    Wf_d = [decl(f"Wf{k}", [D, D], dt.float32, isOutput=False) for k in (2, 3)]
    Wl_d = decl("Wlin", [D, D], dt.bfloat16, isOutput=False)
    Wlf_d = decl("Wlinf", [D, D], dt.float32, isOutput=False)
    We_d = [decl(f"We{k}", [4 * DE, 4 * D], dt.bfloat16, isOutput=False)
            for k in (1, 2, 3)]  # block-diag: 4 copies of We on the diagonal
    # b_tot[k] = b_k + be_k as column [128,1]; rows [1,128] for fold matmuls
    bcol1_d = decl("bcol1", [D, 1], dt.float32, isOutput=False)
    brow_d = [decl(f"brow{k}", [1, D], dt.float32, isOutput=False) for k in (2, 3)]
    blrow_d = decl("blrow", [1, D], dt.float32, isOutput=False)
    g_d = [decl(f"g{k}", [D, 1], dt.float32, isOutput=False) for k in (1, 2, 3)]
    bt_d = [decl(f"bt{k}", [D, 1], dt.float32, isOutput=False) for k in (1, 2, 3)]
    outT = decl("outT", [P, NSH], dt.float32, isOutput=True)
    import os
    DBG = os.environ.get("KDBG") == "1"
    SKIP_GATHER = os.environ.get("KSKIP_GATHER") == "1"
    SKIP_CC = os.environ.get("KSKIP_CC") == "1"
    SKIP_EDGE = os.environ.get("KSKIP_EDGE") == "1"
    SKIP_EA = os.environ.get("KSKIP_EA") == "1"
    if DBG:
        dbg_table = decl("dbg_table", [NP, D], dt.bfloat16, isOutput=True)
        dbg_hr = decl("dbg_hr", [P, 4 * D], dt.bfloat16, isOutput=True)
        dbg_msg = decl("dbg_msg", [P, 4 * D], dt.bfloat16, isOutput=True)
        dbg_t1 = decl("dbg_t1", [P, 6 * P], dt.bfloat16, isOutput=True)
        dbg_stats = decl("dbg_stats", [P, 6], dt.float32, isOutput=True)
        dbg_table2 = decl("dbg_table2", [NP, D], dt.bfloat16, isOutput=True)

    rg = [list(range(NCORES))]

    with tile.TileContext(nc) as tc:
        import contextlib
        with contextlib.ExitStack() as ctx:
            ek = ctx.enter_context
            const = ek(tc.tile_pool(name="const", bufs=1))
            nodeb = ek(tc.tile_pool(name="nodeb", bufs=1))
            edge_ea = ek(tc.tile_pool(name="edge_ea", bufs=3))
            edge_hr = ek(tc.tile_pool(name="edge_hr", bufs=4))
            edge_msg = ek(tc.tile_pool(name="edge_msg", bufs=2))
            edge_S = ek(tc.tile_pool(name="edge_S", bufs=3))
            small = ek(tc.tile_pool(name="small", bufs=4))
            trp = ek(tc.tile_pool(name="trp", bufs=3))
            ps_mp = ek(tc.tile_pool(name="ps_mp", bufs=3, space="PSUM"))
            ps_conv = ek(tc.tile_pool(name="ps_conv", bufs=2, space="PSUM"))
            ps_node = ek(tc.tile_pool(name="ps_node", bufs=1, space="PSUM"))
            ps_tr = ek(tc.tile_pool(name="ps_tr", bufs=1, space="PSUM"))
            dram = ek(tc.tile_pool(name="dram", bufs=2, space="DRAM"))

            # ---- load constants ----
            def ld(pool, shape, dty, src, name):
                t = pool.tile(shape, dty, name=name)
                nc.sync.dma_start(out=t[:], in_=src[...])
                return t

            xT_t = ld(const, [P, NSH], dt.bfloat16, xT, 'xT_t')
            dinv_t = ld(const, [P, NSH], dt.bfloat16, dinv_d, 'dinv_d_t')
            iota_t = ld(const, [P, P], dt.uint8, iota_d, 'iota_d_t')
            ident_t = ld(const, [P, P], dt.bfloat16, ident_d, 'ident_d_t')
            colrel_t = ld(const, [P, TC], dt.uint8, colrel_d, 'colrel_d_t')
            # gather indices are layer-invariant -> SBUF-resident, loaded once
            idx_all = ld(const, [P, TS], dt.int16, idx_d, 'idx_all_t')
            W_t = [ld(const, [D, D], dt.bfloat16, W_d[i], f'W_t{i}') for i in range(3)]
            Wf_t = [ld(const, [D, D], dt.float32, Wf_d[i], f'Wf_t{i}') for i in range(2)]
            Wl_t = ld(const, [D, D], dt.bfloat16, Wl_d, 'Wl_d_t')
            Wlf_t = ld(const, [D, D], dt.float32, Wlf_d, 'Wlf_d_t')
            We_t = [ld(const, [4 * DE, 4 * D], dt.bfloat16, We_d[i], f'We_t{i}')
                    for i in range(3)]
            bcol1_t = ld(const, [D, 1], dt.float32, bcol1_d, 'bcol1_d_t')
            brow_t = [ld(const, [1, D], dt.float32, brow_d[i], f'brow_t{i}') for i in range(2)]
            blrow_t = ld(const, [1, D], dt.float32, blrow_d, 'blrow_d_t')
            g_t = [ld(const, [D, 1], dt.float32, g_d[i], f'g_t{i}') for i in range(3)]
            bt_t = [ld(const, [D, 1], dt.float32, bt_d[i], f'bt_t{i}') for i in range(3)]

            t_T = [const.tile([P, NSH], dt.bfloat16, name=f't_T{i}') for i in range(2)]
            eps_t = const.tile([P, 1], dt.float32, name='eps_t')
            nc.vector.memset(eps_t[:], EPS)

            m_lo, m_hi = sched["m_lo"], sched["m_hi"]
            chunk_off, s_off, s_lo = sched["chunk_off"], sched["s_off"], sched["s_lo"]
            n_grp, grp_off = sched["n_grp"], sched["grp_off"]
            qctr = [0]  # round-robin SWDGE queue for dma_gather

            col_chunks = [(o, min(512, NSH - o)) for o in range(0, NSH, 512)]
            col_chunks_a = [(o, w) for (o, w) in col_chunks if o < PA]
            col_chunks_b = [(o, w) for (o, w) in col_chunks if o >= PA]

            def node_linear(rhs_t, Wp_t, bp_t, hout_t, chunks):
                """hout = dinv * (W'.T @ rhs + b')  (feature-major)."""
                for (o, w) in chunks:
                    pp = ps_node.tile([P, w], dt.float32, space="PSUM",
                                         padded_shape=[P, 512])
                    nc.tensor.matmul(out=pp[:], lhsT=Wp_t[:], rhs=rhs_t[:, o:o + w],
                                     start=True, stop=True)
                    tmp = trp.tile([P, w], dt.bfloat16)
                    nc.scalar.activation(out=tmp[:], in_=pp[:], func=AF.Identity,
                                         bias=bp_t[:])
                    nc.vector.tensor_tensor(out=hout_t[:, o:o + w], in0=tmp[:],
                                            in1=dinv_t[:, o:o + w], op=ALU.mult)

            for k in range(3):  # layers 1..3
                # ---- fold previous BN (k>=1) into this layer's weights ----
                if k == 0:
                    Wp_t, bp_t = W_t[0], bcol1_t
                    rhs_t = xT_t
                else:
                    a_t, c_t = bn_fold  # from previous layer epilogue
                    Wp_t = small.tile([D, D], dt.bfloat16)
                    nc.scalar.activation(out=Wp_t[:], in_=W_t[k][:], func=AF.Identity,
                                         scale=a_t[:])
                    pb = ps_tr.tile([1, D], dt.float32, space="PSUM",
                                    padded_shape=[1, 512])
                    nc.tensor.matmul(out=pb[:], lhsT=c_t[:], rhs=Wf_t[k - 1][:],
                                     start=True, stop=True)
                    bprow = small.tile([1, D], dt.float32)
                    nc.vector.tensor_tensor(out=bprow[:], in0=pb[:], in1=brow_t[k - 1][:],
                                            op=ALU.add)
                    bp_bounce = dram.tile([1, D], dt.float32, name='bp_bounce')
                    nc.sync.dma_start(out=bp_bounce[:], in_=bprow[:])
                    bp_t = small.tile([D, 1], dt.float32)
                    nc.sync.dma_start(out=bp_t[:], in_=bp_bounce[0, :, None])
                    rhs_t = t_T[(k - 1) % 2]

                # ---- node linear + transpose + AllGather, piece by piece ----
                # Piece A's AllGather is issued as soon as its shard is
                # written, so it overlaps piece B's node compute; AG_B then
                # overlaps the early piece-A gathers of the edge phase.
                hlin_t = nodeb.tile([P, NSH], dt.bfloat16)
                shard_a = dram.tile([PA, D], dt.bfloat16)
                shard_b = dram.tile([PB, D], dt.bfloat16)
                table_a = dram.tile([TABA, D], dt.bfloat16)
                table_b = dram.tile([TABB, D], dt.bfloat16)
                BA = PA // P
                for (chunks, t0, t1, shard, table) in (
                    (col_chunks_a, 0, BA, shard_a, table_a),
                    (col_chunks_b, BA, BPC, shard_b, table_b),
                ):
                    node_linear(rhs_t, Wp_t, bp_t, hlin_t, chunks)
                    for t in range(t0, t1):
                        ptr = ps_tr.tile([P, P], dt.bfloat16, space="PSUM",
                                         padded_shape=[P, 1024])
                        nc.tensor.transpose(out=ptr[:], in_=hlin_t[:, t * P:(t + 1) * P],
                                            identity=ident_t[:])
                        sb = trp.tile([P, P], dt.bfloat16)
                        nc.scalar.activation(out=sb[:], in_=ptr[:], func=AF.Copy)
                        nc.sync.dma_start(out=shard[(t - t0) * P:(t - t0 + 1) * P, :],
                                          in_=sb[:])
                    if not SKIP_CC:
                        nc.gpsimd.collective_compute(
                            "AllGather", ALU.bypass, replica_groups=rg,
                            ins=[shard[:].opt()], outs=[table[:].opt()],
                        )
                    else:
                        nc.sync.dma_start(out=table[:shard.shape[0], :], in_=shard[:, :])

                # ---- edge phase over 49 dest blocks ----
                sums_t = small.tile([P, BPC], dt.float32)
                sqs_t = small.tile([P, BPC], dt.float32)
                tnew = t_T[k % 2]
                for b in range(BPC):
                    m = m_lo[b] + m_hi[b]
                    co = chunk_off[b]
                    ng, go = n_grp[b], grp_off[b]
                    ea_t = edge_ea.tile([4 * DE, ng, P], dt.bfloat16)
                    if not SKIP_EA:
                        nc.sync.dma_start(out=ea_t[:], in_=eaT_d[:, go:go + ng, :])
                    else:
                        nc.vector.memset(ea_t[:, :1, :], 0.0)
                    so_b = s_off[b]
                    hr_t = edge_hr.tile([P, m, D], dt.bfloat16)
                    # Gather per (block, table-half), split into balanced
                    # pieces of <=8 chunks (1024 idxs = 65 descs/engine —
                    # the SWDGE ring rejects larger calls; 81 and 137
                    # descs/engine both failed on HW). queue q executes on
                    # Q7 core pair (2q, 2q+1); cycling queues runs 4
                    # gathers' desc-gen concurrently.
                    if not SKIP_GATHER:
                        for mh, tab, mbase, sbase in (
                            (m_lo[b], table_a[:, :], 0, 0),
                            (m_hi[b], table_b[:, :], m_lo[b], s_lo[b]),
                        ):
                            npc = (mh + 7) // 8
                            step = (mh + npc - 1) // npc
                            for pc in range(0, mh, step):
                                pw = min(step, mh - pc)
                                nc.gpsimd.dma_gather(
                                    out_ap=hr_t[:, mbase + pc:mbase + pc + pw, :],
                                    in_ap=tab,
                                    idxs_ap=idx_all[:, so_b + sbase + pc * 8:
                                                    so_b + sbase + (pc + pw) * 8],
                                    num_idxs=pw * P, num_idxs_reg=pw * P, elem_size=D,
                                    queue_num=qctr[0],
                                )
                                qctr[0] = (qctr[0] + 1) % NQ
                    else:
                        nc.vector.memset(hr_t[:, :1, :], 0.25)
                    if DBG and k == 0 and b == 0:
                        dbg_hr_sb = trp.tile([P, 4 * D], dt.bfloat16, name='dbg_hr_sb')
                        nc.vector.tensor_copy(out=dbg_hr_sb[:], in_=hr_t[:, :4, :].rearrange("p m d -> p (m d)"))
                        nc.sync.dma_start(out=dbg_hr[:, :], in_=dbg_hr_sb[:])
                    # S indicator [P, m, P]
                    S_t = edge_S.tile([P, m, P], S_DTYPE)
                    iota_b = bass.AP(tensor=iota_t.tensor, offset=iota_t[:].offset,
                                     ap=[iota_t[:].ap[0], [0, m], iota_t[:].ap[1]])
                    cr = colrel_t[:, co:co + m]
                    cr_b = bass.AP(tensor=colrel_t.tensor, offset=cr.offset,
                                   ap=[cr.ap[0], cr.ap[1], [0, P]])
                    nc.vector.tensor_tensor(out=S_t[:], in0=iota_b, in1=cr_b,
                                            op=ALU.is_equal)
                    # messages: per 4-chunk group, identity-inject hr then one
                    # block-diag We matmul computes e for all 4 chunks.
                    msg_t = edge_msg.tile([P, m, D], dt.bfloat16)
                    if SKIP_EDGE:
                        nc.vector.memset(msg_t[:, :1, :], 0.1)
                    for g in range(ng if not SKIP_EDGE else 0):
                        j = 4 * g
                        jw = min(4, m - j)
                        mp = ps_mp.tile([P, 4, D], dt.float32, space="PSUM")
                        # start=True zeroes the whole 2KB bank, so the
                        # full-tile identity-add must come first.
                        nc.tensor.matmul(
                            out=mp[:, :jw, :].rearrange("p j d -> p (j d)"),
                            lhsT=ident_t[:],
                            rhs=hr_t[:, j:j + jw, :].rearrange("p j d -> p (j d)"),
                            start=True, stop=False, skip_group_check=True)
                        nc.tensor.matmul(
                            out=mp[:, :jw, :].rearrange("p j d -> p (j d)"),
                            lhsT=ea_t[:16 * jw, g, :],
                            rhs=We_t[k][:16 * jw, :jw * D],
                            start=False, stop=True, skip_group_check=True)
                        nc.scalar.activation(
                            out=msg_t[:, j:j + jw, :].rearrange("p j d -> p (j d)"),
                            in_=mp[:, :jw, :].rearrange("p j d -> p (j d)"), func=AF.Relu)
                    if DBG and k == 0 and b == 0:
                        dbg_msg_sb = trp.tile([P, 4 * D], dt.bfloat16, name='dbg_msg_sb')
                        nc.vector.tensor_copy(out=dbg_msg_sb[:], in_=msg_t[:, :4, :].rearrange("p m d -> p (m d)"))
                        nc.sync.dma_start(out=dbg_msg[:, :], in_=dbg_msg_sb[:])
                    # scatter into conv accumulator (feature-major out)
                    cp = ps_conv.tile([P, P], dt.float32, space="PSUM",
                                      padded_shape=[P, 512])
                    for j in range(m if not SKIP_EDGE else 1):
                        nc.tensor.matmul(out=cp[:], lhsT=msg_t[:, j, :],
                                         rhs=S_t[:, j, :],
                                         start=(j == 0), stop=(j == m - 1))
                    # epilogue: dinv scale, relu -> t, stats
                    sl = slice(b * P, (b + 1) * P)
                    pre = trp.tile([P, P], dt.float32)
                    nc.vector.tensor_tensor(out=pre[:], in0=cp[:],
                                            in1=dinv_t[:, sl], op=ALU.mult)
                    nc.scalar.activation(out=tnew[:, sl], in_=pre[:], func=AF.Relu,
                                         accum_out=sums_t[:, b:b + 1])
                    sq_scr = trp.tile([P, P], dt.bfloat16)
                    nc.scalar.activation(out=sq_scr[:], in_=tnew[:, sl], func=AF.Square,
                                         accum_out=sqs_t[:, b:b + 1])

                # ---- BN stats + fold coefficients ----
                st = small.tile([P, 2], dt.float32)
                nc.vector.tensor_reduce(out=st[:, 0:1], in_=sums_t[:],
                                        axis=mybir.AxisListType.X, op=ALU.add)
                nc.vector.tensor_reduce(out=st[:, 1:2], in_=sqs_t[:],
                                        axis=mybir.AxisListType.X, op=ALU.add)
                st_in = dram.tile([P, 2], dt.float32)
                st_out = dram.tile([P, 2], dt.float32)
                nc.sync.dma_start(out=st_in[:], in_=st[:])
                if not SKIP_CC:
                    nc.gpsimd.collective_compute(
                        "AllReduce", ALU.add, replica_groups=rg,
                        ins=[st_in[:].opt()], outs=[st_out[:].opt()],
                    )
                else:
                    nc.sync.dma_start(out=st_out[:, :], in_=st_in[:, :])
                stg = small.tile([P, 2], dt.float32)
                nc.sync.dma_start(out=stg[:], in_=st_out[:])
                mu = small.tile([P, 1], dt.float32)
                nc.vector.tensor_scalar(out=mu[:], in0=stg[:, 0:1], scalar1=1.0 / N,
                                        scalar2=None, op0=ALU.mult)
                ex2 = small.tile([P, 1], dt.float32)
                nc.vector.tensor_scalar(out=ex2[:], in0=stg[:, 1:2], scalar1=1.0 / N,
                                        scalar2=None, op0=ALU.mult)
                var = small.tile([P, 1], dt.float32)
                nc.vector.tensor_tensor(out=var[:], in0=mu[:], in1=mu[:], op=ALU.mult)
                nc.vector.tensor_tensor(out=var[:], in0=ex2[:], in1=var[:],
                                        op=ALU.subtract)
                sd = small.tile([P, 1], dt.float32)
                nc.scalar.activation(out=sd[:], in_=var[:], func=AF.Sqrt, bias=eps_t[:])
                rs = small.tile([P, 1], dt.float32)
                nc.vector.reciprocal(out=rs[:], in_=sd[:])
                a_t = small.tile([P, 1], dt.float32)
                nc.vector.tensor_tensor(out=a_t[:], in0=rs[:], in1=g_t[k][:],
                                        op=ALU.mult)
                c_t = small.tile([P, 1], dt.float32)
                nc.vector.tensor_tensor(out=c_t[:], in0=mu[:], in1=a_t[:], op=ALU.mult)
                nc.vector.tensor_tensor(out=c_t[:], in0=bt_t[k][:], in1=c_t[:],
                                        op=ALU.subtract)
                bn_fold = (a_t, c_t)
                if DBG and k == 0:
                    nt = min(6 * P, NSH)
                    dbg_t1_sb = trp.tile([P, 6 * P], dt.bfloat16, name='dbg_t1_sb')
                    nc.vector.tensor_copy(out=dbg_t1_sb[:, :nt], in_=tnew[:, :nt])
                    nc.sync.dma_start(out=dbg_t1[:, :], in_=dbg_t1_sb[:])
                    dbg_st_sb = trp.tile([P, 6], dt.float32, name='dbg_st_sb')
                    for ii, tt in enumerate((mu, ex2, var, sd, a_t, c_t)):
                        nc.vector.tensor_copy(out=dbg_st_sb[:, ii:ii+1], in_=tt[:])
                    nc.sync.dma_start(out=dbg_stats[:, :], in_=dbg_st_sb[:])

            # ---- final linear: out^T = Wl'.T @ t3 + bl' ----
            a_t, c_t = bn_fold
            Wlp = small.tile([D, D], dt.bfloat16)
            nc.scalar.activation(out=Wlp[:], in_=Wl_t[:], func=AF.Identity, scale=a_t[:])
            pb = ps_tr.tile([1, D], dt.float32, space="PSUM",
                            padded_shape=[1, 512])
            nc.tensor.matmul(out=pb[:], lhsT=c_t[:], rhs=Wlf_t[:], start=True, stop=True)
            blp_row = small.tile([1, D], dt.float32)
            nc.vector.tensor_tensor(out=blp_row[:], in0=pb[:], in1=blrow_t[:], op=ALU.add)
            blp_bounce = dram.tile([1, D], dt.float32, name='blp_bounce')
            nc.sync.dma_start(out=blp_bounce[:], in_=blp_row[:])
            blp = small.tile([D, 1], dt.float32)
            nc.sync.dma_start(out=blp[:], in_=blp_bounce[0, :, None])
            for (o, w) in col_chunks:
                pp = ps_node.tile([P, w], dt.float32, space="PSUM",
                                     padded_shape=[P, 512])
                nc.tensor.matmul(out=pp[:], lhsT=Wlp[:], rhs=t_T[0][:, o:o + w],
                                 start=True, stop=True)
                ot = trp.tile([P, w], dt.float32)
                nc.scalar.activation(out=ot[:], in_=pp[:], func=AF.Identity, bias=blp[:])
                nc.sync.dma_start(out=outT[:, o:o + w], in_=ot[:])

    nc.finalize()
    return nc


# ---------------- public entry point ----------------

_CACHE = {}
LAST_EXEC_NS = None


def _make_in_maps(inputs, per_core, dinv):
    x = np.asarray(inputs["x"], np.float32)

    dinv_pad = np.zeros(NP, np.float32)
    dinv_pad[:N] = dinv
    xT_full = np.zeros((P, NP), np.float32)
    xT_full[:, :N] = x.T

    Ws = {k: np.asarray(inputs[k], np.float32) for k in
          ("W1", "W2", "W3", "Wl", "We1", "We2", "We3")}
    bt_tot = {k: np.asarray(inputs[f"b{k}"], np.float32) +
                 np.asarray(inputs[f"be{k}"], np.float32) for k in (1, 2, 3)}

    in_maps = []
    for c in range(NCORES):
        sl = slice(c * NSH, (c + 1) * NSH)
        im = dict(per_core[c])
        im["xT"] = _bf16(xT_full[:, sl])
        im["dinvt"] = _bf16(np.tile(dinv_pad[sl][None, :], (P, 1)))
        im["iota_u8"] = np.tile(np.arange(P, dtype=np.uint8)[None, :], (P, 1))
        im["ident"] = _bf16(np.eye(P))
        for i, k in enumerate((1, 2, 3)):
            im[f"W{k}"] = _bf16(Ws[f"W{k}"])
            Webd = np.zeros((4 * DE, 4 * D), np.float32)
            for j in range(4):
                Webd[16 * j:16 * j + 16, D * j:D * j + D] = Ws[f"We{k}"]
            im[f"We{k}"] = _bf16(Webd)
            im[f"g{k}"] = np.asarray(inputs[f"g{k}"], np.float32).reshape(D, 1)
            im[f"bt{k}"] = np.asarray(inputs[f"bt{k}"], np.float32).reshape(D, 1)
        im["Wf2"] = Ws["W2"]
        im["Wf3"] = Ws["W3"]
        im["Wlin"] = _bf16(Ws["Wl"])
        im["Wlinf"] = Ws["Wl"]
        im["bcol1"] = bt_tot[1].reshape(D, 1)
        im["brow2"] = bt_tot[2].reshape(1, D)
        im["brow3"] = bt_tot[3].reshape(1, D)
        im["blrow"] = np.asarray(inputs["bl"], np.float32).reshape(1, D)
        in_maps.append(im)
    return in_maps


def kernel(**inputs):
    edge_attr = np.asarray(inputs["edge_attr"], np.float32)
    edge_index = np.asarray(inputs["edge_index"])

    per_core, sched, dinv = _preprocess(edge_index, edge_attr)
    in_maps = _make_in_maps(inputs, per_core, dinv)

    key = ("k", sched["tot_chunks"], sched["tot_s"],
           tuple(sched["m_lo"]), tuple(sched["m_hi"]))
    if key not in _CACHE:
        _CACHE[key] = _build(sched)
    nc = _CACHE[key]

    import os
    trace = os.environ.get("KPROF") == "1"
    r = run_bass_kernel_spmd(nc, in_maps, core_ids=list(range(NCORES)), trace=trace)
    if trace:
        print(f"HW exec time: {r.exec_time_ns} ns", flush=True)
        global LAST_EXEC_NS
        LAST_EXEC_NS = r.exec_time_ns
        try:
            insts = r.instructions_and_trace[0] if r.instructions_and_trace else []
            import pickle
            rows = [
                dict(ts=i.timestamp, dur=i.duration, engine=str(i.engine),
                     name=i.name, label=i.label, wait=i.evt_wait_time,
                     bir=str(i.bir_str)[:200], src=f"{i.source_file}:{i.source_line}")
                for i in insts
            ]
            with open("/tmp/kprof_insts.pkl", "wb") as f:
                pickle.dump(rows, f)
            print(f"KPROF: dumped {len(rows)} insts; trace="
                  f"{r.instructions_and_trace[1] if r.instructions_and_trace else None}",
                  flush=True)
        except Exception as e:
            print(f"KPROF dump failed: {e}", flush=True)
    res = r.results
    outT = np.concatenate([res[c]["outT"] for c in range(NCORES)], axis=1)  # [128, NP]
    return np.ascontiguousarray(outT.T[:N]).astype(np.float32)

